# revision 1
# baseline (speedup 1.0000x reference)
"""Trainium2 Bass kernel for ConvReverseDataNet (USRNet-style FFT data step).

Math (per (b,c) plane, sf=2, validated vs reference in fp32):
  g   = fft2_128(x)                                (128x128 complex)
  FB  = G @ k @ G^T, G = F256[:, roll_idx]         (256x256 complex)
  W   = sum_{4 blocks} |FB|^2 ;  Y0 = sum_{4 blocks} FB*DD   (128x128)
  wt  = (4 - Y0) / (W + 4*be)                      (128x128 complex)
  FX  = tile(g) * (conj(FB)*tile(wt) + DD)         (256x256 complex)
  out = real(ifft2_256(FX))                         = Fc@FX@Fc / 65536
where DD = outer(d, d), d[t] = 1 + exp(-2*pi*i*t/256), be = sigmoid(alpha-9)+1e-3.
All complex arrays X are stored as (Xr, Xs) with X = Xr - i*Xs.
256x256 planes live in SBUF as [128, 512]: [p, hb*256+f] = plane[hb*128+p, f].

Sharding: 256 (b,c) planes over 8 cores; core ci gets channels ci*8..ci*8+7 x all 4 batches.
"""

import functools
import sys

import numpy as np

if "/opt/trn_rl_repo" not in sys.path:
    sys.path.insert(0, "/opt/trn_rl_repo")

from concourse import bacc, bass, mybir, tile  # noqa: E402
from concourse.bass_utils import run_bass_kernel_spmd  # noqa: E402

F32 = mybir.dt.float32
MULT = mybir.AluOpType.mult
ADD = mybir.AluOpType.add

N_CORES = 8
NPL = 32  # planes per core
KS = 25


def _host_consts():
    t1 = np.arange(128)
    th1 = 2 * np.pi * np.outer(t1, t1) / 128
    C1 = np.cos(th1).astype(np.float32)
    S1 = np.sin(th1).astype(np.float32)
    t2 = np.arange(256)
    th2 = 2 * np.pi * np.outer(t2, t2) / 256
    C2 = np.cos(th2)
    S2 = np.sin(th2)
    idx = (np.arange(KS) - (KS // 2)) % 256
    GcT = C2[idx, :].astype(np.float32)  # [25,256]
    GsT = S2[idx, :].astype(np.float32)
    # Cnat[p, kc*256+m] = C2[kc*128+p, m]
    Cnat = C2.reshape(2, 128, 256).transpose(1, 0, 2).reshape(128, 512).astype(np.float32)
    Snat = S2.reshape(2, 128, 256).transpose(1, 0, 2).reshape(128, 512).astype(np.float32)
    dr = 1 + np.cos(2 * np.pi * t2 / 256)
    ds = np.sin(2 * np.pi * t2 / 256)

    def to_plane(a):
        return a.reshape(2, 128, 256).transpose(1, 0, 2).reshape(128, 512)

    DDr = to_plane(np.outer(dr, dr) - np.outer(ds, ds)).astype(np.float32)
    DDs = to_plane(np.outer(dr, ds) + np.outer(ds, dr)).astype(np.float32)
    return {
        "C1": C1, "S1": S1, "S1n": -S1,
        "GcT": GcT, "GsT": GsT, "GsTn": -GsT,
        "Cnat": Cnat, "Snat": Snat, "Snatn": -Snat,
        "DDr": DDr, "DDs": DDs,
    }


CONST_SHAPES = {
    "C1": [128, 128], "S1": [128, 128], "S1n": [128, 128],
    "GcT": [KS, 256], "GsT": [KS, 256], "GsTn": [KS, 256],
    "Cnat": [128, 512], "Snat": [128, 512], "Snatn": [128, 512],
    "DDr": [128, 512], "DDs": [128, 512],
}


def build_nc(n_planes=NPL):
    nc = bacc.Bacc("TRN2", target_bir_lowering=False, debug=False, num_devices=N_CORES)

    xs_t = nc.dram_tensor("xs", [n_planes, 128, 128], F32, kind="ExternalInput")
    kt_t = nc.dram_tensor("kt", [n_planes, KS, KS], F32, kind="ExternalInput")
    be4_t = nc.dram_tensor("be4", [128, n_planes], F32, kind="ExternalInput")
    const_t = {n: nc.dram_tensor(n, s, F32, kind="ExternalInput") for n, s in CONST_SHAPES.items()}
    out_t = nc.dram_tensor("out", [n_planes, 256, 256], F32, kind="ExternalOutput")

    with tile.TileContext(nc) as tc:
        with (
            tc.tile_pool(name="cpool", bufs=1) as cpool,
            tc.tile_pool(name="small", bufs=3) as small,
            tc.tile_pool(name="big", bufs=2) as big,
            tc.tile_pool(name="psum", bufs=1, space="PSUM") as pp,
        ):
            cs = {}
            for n, s in CONST_SHAPES.items():
                cs[n] = cpool.tile(s, F32, tag=n, name=f"c_{n}")
                nc.sync.dma_start(cs[n][:], const_t[n][:])
            be4sb = cpool.tile([128, n_planes], F32, tag="be4sb")
            nc.sync.dma_start(be4sb[:], be4_t[:])

            def b4(ap):  # view flat [128,512] as [128,4,128]
                return ap.rearrange("p (b f) -> p b f", b=4)

            def rep4(ap128):  # [128,128] -> broadcast [128,4,128]
                return ap128.unsqueeze(1).broadcast_to([128, 4, 128])

            for i in range(n_planes):
                # ---- loads ----
                x_sb = small.tile([128, 128], F32, tag="x_sb")
                nc.sync.dma_start(x_sb[:], xs_t[i])
                kt_sb = small.tile([KS, KS], F32, tag="kt_sb")
                nc.sync.dma_start(kt_sb[:], kt_t[i])

                # ---- fft128: g = F1 @ x @ F1 ----
                z_sb = small.tile([128, 256], F32, tag="z_sb")  # Zrt | Zst
                pzr = pp.tile([128, 128], F32, tag="p128", bufs=2)
                nc.tensor.matmul(pzr[:], x_sb[:], cs["C1"][:], start=True, stop=True)
                nc.scalar.copy(z_sb[:, 0:128], pzr[:])
                pzs = pp.tile([128, 128], F32, tag="p128", bufs=2)
                nc.tensor.matmul(pzs[:], x_sb[:], cs["S1"][:], start=True, stop=True)
                nc.scalar.copy(z_sb[:, 128:256], pzs[:])

                g_sb = small.tile([128, 256], F32, tag="g_sb")  # gr | gs
                pgr = pp.tile([128, 128], F32, tag="p128", bufs=2)
                nc.tensor.matmul(pgr[:], z_sb[:, 0:128], cs["C1"][:], start=True, stop=False)
                nc.tensor.matmul(pgr[:], z_sb[:, 128:256], cs["S1n"][:], start=False, stop=True)
                nc.scalar.copy(g_sb[:, 0:128], pgr[:])
                pgs = pp.tile([128, 128], F32, tag="p128", bufs=2)
                nc.tensor.matmul(pgs[:], z_sb[:, 0:128], cs["S1"][:], start=True, stop=False)
                nc.tensor.matmul(pgs[:], z_sb[:, 128:256], cs["C1"][:], start=False, stop=True)
                nc.scalar.copy(g_sb[:, 128:256], pgs[:])

                # ---- FB = G @ k @ G^T ----
                a_sb = small.tile([KS, 512], F32, tag="a_sb")  # Ar | As
                par = pp.tile([KS, 256], F32, tag="pa")
                nc.tensor.matmul(par[:], kt_sb[:], cs["GcT"][:], start=True, stop=True)
                nc.scalar.copy(a_sb[:, 0:256], par[:])
                pas = pp.tile([KS, 256], F32, tag="pa")
                nc.tensor.matmul(pas[:], kt_sb[:], cs["GsT"][:], start=True, stop=True)
                nc.scalar.copy(a_sb[:, 256:512], pas[:])

                FBr = big.tile([128, 512], F32, tag="FBr")
                FBs = big.tile([128, 512], F32, tag="FBs")
                for hh in range(2):
                    hsl = slice(hh * 128, (hh + 1) * 128)
                    pfbr = pp.tile([128, 256], F32, tag="pfb", bufs=2)
                    nc.tensor.matmul(pfbr[:], cs["GcT"][:, hsl], a_sb[:, 0:256], start=True, stop=False)
                    nc.tensor.matmul(pfbr[:], cs["GsTn"][:, hsl], a_sb[:, 256:512], start=False, stop=True)
                    nc.scalar.copy(FBr[:, hh * 256:(hh + 1) * 256], pfbr[:])
                    pfbs = pp.tile([128, 256], F32, tag="pfb", bufs=2)
                    nc.tensor.matmul(pfbs[:], cs["GcT"][:, hsl], a_sb[:, 256:512], start=True, stop=False)
                    nc.tensor.matmul(pfbs[:], cs["GsT"][:, hsl], a_sb[:, 0:256], start=False, stop=True)
                    nc.scalar.copy(FBs[:, hh * 256:(hh + 1) * 256], pfbs[:])

                # ---- elementwise: W, Y0, wt ----
                sq1 = big.tile([128, 512], F32, tag="sq1")
                sq2 = big.tile([128, 512], F32, tag="sq2")
                nc.scalar.square(sq1[:], FBr[:])
                nc.scalar.square(sq2[:], FBs[:])
                F2B = big.tile([128, 512], F32, tag="F2B")
                nc.vector.tensor_add(F2B[:], sq1[:], sq2[:])

                m1 = big.tile([128, 512], F32, tag="m1")
                m2 = big.tile([128, 512], F32, tag="m2")
                nc.vector.tensor_mul(m1[:], FBr[:], cs["DDr"][:])
                nc.vector.scalar_tensor_tensor(m2[:], FBs[:], -1.0, cs["DDs"][:], MULT, MULT)
                Pr0 = big.tile([128, 512], F32, tag="Pr0")
                nc.vector.tensor_add(Pr0[:], m1[:], m2[:])
                m3 = big.tile([128, 512], F32, tag="m3")
                m4 = big.tile([128, 512], F32, tag="m4")
                nc.vector.tensor_mul(m3[:], FBr[:], cs["DDs"][:])
                nc.vector.tensor_mul(m4[:], FBs[:], cs["DDr"][:])
                Ps0 = big.tile([128, 512], F32, tag="Ps0")
                nc.vector.tensor_add(Ps0[:], m3[:], m4[:])

                sums = {}
                for nm, src in (("W", F2B), ("Yr0", Pr0), ("Ys0", Ps0)):
                    sA = small.tile([128, 256], F32, tag=f"sA_{nm}")
                    v = src[:].rearrange("p (a b f) -> p a b f", a=2, b=2)
                    nc.vector.tensor_add(sA[:].rearrange("p (a f) -> p a f", a=2), v[:, :, 0, :], v[:, :, 1, :])
                    dst = small.tile([128, 128], F32, tag=nm)
                    nc.vector.tensor_add(dst[:], sA[:, 0:128], sA[:, 128:256])
                    sums[nm] = dst

                den = small.tile([128, 128], F32, tag="den")
                nc.vector.tensor_scalar_add(den[:], sums["W"][:], be4sb[:, i:i + 1])
                dinv = small.tile([128, 128], F32, tag="dinv")
                nc.vector.reciprocal_approx_fast(dinv[:], den[:])
                wt4 = small.tile([128, 128], F32, tag="wt4")
                nc.vector.tensor_scalar(wt4[:], sums["Yr0"][:], -1.0, 4.0, MULT, ADD)
                wr = small.tile([128, 128], F32, tag="wr")
                nc.vector.tensor_mul(wr[:], wt4[:], dinv[:])
                ws = small.tile([128, 128], F32, tag="ws")
                nc.vector.scalar_tensor_tensor(ws[:], sums["Ys0"][:], -1.0, dinv[:], MULT, MULT)

                # ---- H = conj(FB) * tile(wt) + DD ----
                twr = rep4(wr[:])
                tws = rep4(ws[:])
                p1 = big.tile([128, 512], F32, tag="p1")
                p2 = big.tile([128, 512], F32, tag="p2")
                nc.vector.tensor_mul(b4(p1[:]), b4(FBr[:]), twr)
                nc.vector.tensor_mul(b4(p2[:]), b4(FBs[:]), tws)
                s12 = big.tile([128, 512], F32, tag="s12")
                nc.vector.tensor_add(s12[:], p1[:], p2[:])
                Hr = big.tile([128, 512], F32, tag="Hr")
                nc.vector.tensor_add(Hr[:], s12[:], cs["DDr"][:])
                p3 = big.tile([128, 512], F32, tag="p3")
                p4 = big.tile([128, 512], F32, tag="p4")
                nc.vector.tensor_mul(b4(p3[:]), b4(FBr[:]), tws)
                nc.vector.scalar_tensor_tensor(b4(p4[:]), b4(FBs[:]), -1.0, twr, MULT, MULT)
                s34 = big.tile([128, 512], F32, tag="s34")
                nc.vector.tensor_add(s34[:], p3[:], p4[:])
                Hs = big.tile([128, 512], F32, tag="Hs")
                nc.vector.tensor_add(Hs[:], s34[:], cs["DDs"][:])

                # ---- FX = tile(g) * H ----
                tgr = rep4(g_sb[:, 0:128])
                tgs = rep4(g_sb[:, 128:256])
                q1 = big.tile([128, 512], F32, tag="q1")
                q2 = big.tile([128, 512], F32, tag="q2")
                nc.vector.tensor_mul(b4(q1[:]), b4(Hr[:]), tgr)
                nc.vector.scalar_tensor_tensor(b4(q2[:]), b4(Hs[:]), -1.0, tgs, MULT, MULT)
                FXr = big.tile([128, 512], F32, tag="FXr")
                nc.vector.tensor_add(FXr[:], q1[:], q2[:])
                q3 = big.tile([128, 512], F32, tag="q3")
                q4 = big.tile([128, 512], F32, tag="q4")
                nc.vector.tensor_mul(b4(q3[:]), b4(Hs[:]), tgr)
                nc.vector.tensor_mul(b4(q4[:]), b4(Hr[:]), tgs)
                FXs = big.tile([128, 512], F32, tag="FXs")
                nc.vector.tensor_add(FXs[:], q3[:], q4[:])

                # ---- ifft stage 1: VT ----
                VTr = big.tile([128, 512], F32, tag="VTr")
                VTs = big.tile([128, 512], F32, tag="VTs")
                for fb in range(2):
                    pvtr = pp.tile([128, 256], F32, tag="pvt", bufs=2)
                    pvts = pp.tile([128, 256], F32, tag="pvt", bufs=2)
                    for kc in range(2):
                        lsl = slice(kc * 256 + fb * 128, kc * 256 + (fb + 1) * 128)
                        csl = slice(kc * 256, (kc + 1) * 256)
                        st = kc == 0
                        nc.tensor.matmul(pvtr[:], FXr[:, lsl], cs["Cnat"][:, csl], start=st, stop=False)
                        nc.tensor.matmul(pvtr[:], FXs[:, lsl], cs["Snat"][:, csl], start=False, stop=(kc == 1))
                    for kc in range(2):
                        lsl = slice(kc * 256 + fb * 128, kc * 256 + (fb + 1) * 128)
                        csl = slice(kc * 256, (kc + 1) * 256)
                        st = kc == 0
                        nc.tensor.matmul(pvts[:], FXs[:, lsl], cs["Cnat"][:, csl], start=st, stop=False)
                        nc.tensor.matmul(pvts[:], FXr[:, lsl], cs["Snatn"][:, csl], start=False, stop=(kc == 1))
                    nc.scalar.copy(VTr[:, fb * 256:(fb + 1) * 256], pvtr[:])
                    nc.scalar.copy(VTs[:, fb * 256:(fb + 1) * 256], pvts[:])

                # ---- ifft stage 2 (real part) + scale + store ----
                out_sb = big.tile([128, 512], F32, tag="out_sb")
                po = pp.tile([128, 512], F32, tag="po")
                for mb in range(2):
                    osl = slice(mb * 256, (mb + 1) * 256)
                    for fb in range(2):
                        lsl = slice(fb * 256 + mb * 128, fb * 256 + (mb + 1) * 128)
                        csl = slice(fb * 256, (fb + 1) * 256)
                        nc.tensor.matmul(po[:, osl], VTr[:, lsl], cs["Cnat"][:, csl], start=(fb == 0), stop=False)
                        nc.tensor.matmul(po[:, osl], VTs[:, lsl], cs["Snat"][:, csl], start=False, stop=(fb == 1))
                    nc.scalar.mul(out_sb[:, osl], po[:, osl], 1.0 / 65536.0)
                nc.sync.dma_start(
                    out_t[i].rearrange("(hb p) f -> p hb f", p=128),
                    out_sb[:].rearrange("p (hb f) -> p hb f", hb=2),
                )

    nc.compile()
    return nc


@functools.lru_cache(maxsize=2)
def _built(n_planes=NPL):
    return build_nc(n_planes)


def make_in_maps(x, k, alpha, n_planes=NPL, n_cores=N_CORES):
    consts = _host_consts()
    alpha_c = alpha.reshape(-1).astype(np.float64)  # [64]
    be = (1.0 / (1.0 + np.exp(-(alpha_c - 9.0))) + 1e-3).astype(np.float32)
    cpc = n_planes // 4  # channels per core
    in_maps = []
    for ci in range(n_cores):
        chs = slice(ci * cpc, (ci + 1) * cpc)
        xs = np.ascontiguousarray(x[:, chs].transpose(1, 0, 2, 3).reshape(n_planes, 128, 128))
        kt = np.ascontiguousarray(k[:, chs].transpose(1, 0, 3, 2).reshape(n_planes, KS, KS))
        be_pl = np.repeat(be[chs], 4)  # plane order: (c_loc, b)
        be4 = np.broadcast_to(4.0 * be_pl, (128, n_planes)).astype(np.float32).copy()
        m = {"xs": xs, "kt": kt, "be4": be4}
        m.update(consts)
        in_maps.append(m)
    return in_maps


def kernel(x, k, alpha, sf=2, **_ignored):
    x = np.asarray(x, dtype=np.float32)
    k = np.asarray(k, dtype=np.float32)
    alpha = np.asarray(alpha, dtype=np.float32)
    assert int(sf) == 2 and x.shape == (4, 64, 128, 128) and k.shape == (4, 64, KS, KS)

    nc = _built(NPL)
    in_maps = make_in_maps(x, k, alpha)
    res = run_bass_kernel_spmd(nc, in_maps, core_ids=list(range(N_CORES)))
    out = np.empty((4, 64, 256, 256), np.float32)
    cpc = NPL // 4
    for ci in range(N_CORES):
        o = res.results[ci]["out"].reshape(cpc, 4, 256, 256).transpose(1, 0, 2, 3)
        out[:, ci * cpc:(ci + 1) * cpc] = o
    return out


if __name__ == "__main__":
    rng = np.random.default_rng(0)
    x = rng.standard_normal((4, 64, 128, 128), dtype=np.float32)
    k = rng.random((4, 64, KS, KS), dtype=np.float32)
    alpha = np.zeros((1, 64, 1, 1), np.float32)
    out = kernel(x, k, alpha, 2)
    print("out", out.shape, out.dtype, float(np.abs(out).max()))



# revision 8
# speedup vs baseline: 2.2069x; 2.2069x over previous
"""Trainium2 Bass kernel for ConvReverseDataNet (USRNet-style FFT data step), v2.

Math per (b,c) plane (sf=2), storage convention X = Xr - i*Xs:
  g   = fft2_128(x)
  FB  = G k G^T, G = F256[:, roll_idx]            (256x256, as [128,1024] r|s)
  W   = blocksum|FB|^2 (128x128); Y0 = Gd k Gd^T  (= blocksum(FB*DD))
  wt  = (4 - Y0) / (W + 4*be)
  gw  = g * wt
  FX  = conj(FB) * tile(gw)
  out = real(ifft2_256(FX)) + nn_upsample(x)      (upsample via matmul w/ Prow)
where Gd[u0,:] = sum_a d_a[u0] * G[a*128+u0,:], d = 1+exp(-2pi i t/256) halves.

Engine split: PE fp32r matmuls (ifft stage1 in bf16), DVE bf16 elementwise
(2x_1p), Act PSUM->SBUF copies, GpSimd(Pool) wt/gw small ops (SBUF only).
Sharding: 256 (b,c) planes over 8 cores; core ci gets channels ci*8..ci*8+7.
"""

import functools
import sys

import numpy as np

if "/opt/trn_rl_repo" not in sys.path:
    sys.path.insert(0, "/opt/trn_rl_repo")

from concourse import bacc, bass, mybir, tile  # noqa: E402
from concourse.bass_utils import run_bass_kernel_spmd  # noqa: E402

F32 = mybir.dt.float32
F32R = mybir.dt.float32r
BF16 = mybir.dt.bfloat16
MULT = mybir.AluOpType.mult
ADD = mybir.AluOpType.add
SUB = mybir.AluOpType.subtract

N_CORES = 8
NPL = 32  # planes per core
KS = 25


def _host_consts():
    t1 = np.arange(128)
    th1 = 2 * np.pi * np.outer(t1, t1) / 128
    C1 = np.cos(th1)
    S1 = np.sin(th1)
    t2 = np.arange(256)
    th2 = 2 * np.pi * np.outer(t2, t2) / 256
    C2 = np.cos(th2)
    S2 = np.sin(th2)
    idx = (np.arange(KS) - (KS // 2)) % 256
    GcT = C2[idx, :]  # [25,256]
    GsT = S2[idx, :]
    th = 2 * np.pi * t1 / 256
    dr0 = 1 + np.cos(th)
    ds0 = np.sin(th)
    dr1 = 1 - np.cos(th)
    ds1 = -np.sin(th)
    Gc0, Gc1 = GcT[:, :128].T, GcT[:, 128:].T
    Gs0, Gs1 = GsT[:, :128].T, GsT[:, 128:].T
    Gdr = dr0[:, None] * Gc0 - ds0[:, None] * Gs0 + dr1[:, None] * Gc1 - ds1[:, None] * Gs1
    Gds = dr0[:, None] * Gs0 + ds0[:, None] * Gc0 + dr1[:, None] * Gs1 + ds1[:, None] * Gc1
    Cnat = C2.reshape(2, 128, 256).transpose(1, 0, 2).reshape(128, 512)
    Snat = S2.reshape(2, 128, 256).transpose(1, 0, 2).reshape(128, 512)
    Prow = np.zeros((2, 128, 128))
    for mb in range(2):
        for m in range(128):
            Prow[mb, mb * 64 + m // 2, m] = 1.0
    f32 = lambda a: np.ascontiguousarray(a, dtype=np.float32)
    return {
        "F1cs": f32(np.concatenate([C1, S1], 1)),       # [128,256]
        "F1b": f32(np.concatenate([-S1, C1], 1)),       # [128,256]
        "GG": f32(np.concatenate([GcT, GsT], 1)),       # [25,512]
        "GsTn": f32(-GsT),                              # [25,256]
        "GDa": f32(np.concatenate([Gdr.T, Gds.T], 1)),  # [25,256]
        "GDb": f32(np.concatenate([-Gds.T, Gdr.T], 1)),  # [25,256]
        "CnatF": f32(Cnat),                             # [128,512]
        "SnatF": f32(Snat),                             # [128,512]
        "Prow0": f32(Prow[0]),                          # [128,128]
        "Prow1": f32(Prow[1]),                          # [128,128]
    }


CONST_SHAPES = {
    "F1cs": [128, 256], "F1b": [128, 256],
    "GG": [25, 512], "GsTn": [25, 256],
    "GDa": [25, 256], "GDb": [25, 256],
    "CnatF": [128, 512], "SnatF": [128, 512],
    "Prow0": [128, 128], "Prow1": [128, 128],
}


def build_nc(n_planes=NPL):
    nc = bacc.Bacc("TRN2", target_bir_lowering=False, debug=False, num_devices=N_CORES)

    xs_t = nc.dram_tensor("xs", [n_planes, 128, 128], F32R, kind="ExternalInput")
    kt_t = nc.dram_tensor("kt", [n_planes, KS, KS], F32R, kind="ExternalInput")
    be4_t = nc.dram_tensor("be4", [128, n_planes], F32, kind="ExternalInput")
    const_t = {n: nc.dram_tensor(n, s, (F32 if n in ("CnatF", "SnatF") else F32R), kind="ExternalInput") for n, s in CONST_SHAPES.items()}
    out_t = nc.dram_tensor("out", [n_planes, 256, 256], F32, kind="ExternalOutput")

    with tile.TileContext(nc) as tc:
        with (
            tc.tile_pool(name="cpool", bufs=1) as cpool,
            tc.tile_pool(name="small", bufs=3) as small,
            tc.tile_pool(name="big", bufs=2) as big,
            tc.tile_pool(name="psS", bufs=2, space="PSUM") as psS,   # [128,256] Z/G/Y
            tc.tile_pool(name="psA", bufs=1, space="PSUM") as psA,   # [25,512] A/A2
            tc.tile_pool(name="psB", bufs=2, space="PSUM") as psB,   # [128,512] FB/VT
            tc.tile_pool(name="psO", bufs=1, space="PSUM") as psO,   # [128,512] po
        ):
            cs = {}
            for n, s in CONST_SHAPES.items():
                cs[n] = cpool.tile(s, (F32 if n in ("CnatF", "SnatF") else F32R), tag=n, name=f"c_{n}")
                nc.sync.dma_start(cs[n][:], const_t[n][:])
            be4sb = cpool.tile([128, n_planes], F32, tag="be4sb")
            nc.sync.dma_start(be4sb[:], be4_t[:])

            # preamble: derived consts (bf16 stage-1 DFT; scaled f32 stage-2 DFT)
            Cn1 = cpool.tile([128, 512], BF16, tag="Cn1")
            Sn1 = cpool.tile([128, 512], BF16, tag="Sn1")
            Sn1n = cpool.tile([128, 512], BF16, tag="Sn1n")
            Cn2 = cpool.tile([128, 512], F32R, tag="Cn2")
            Sn2 = cpool.tile([128, 512], F32R, tag="Sn2")
            nc.scalar.copy(Cn1[:], cs["CnatF"][:])
            nc.scalar.copy(Sn1[:], cs["SnatF"][:])
            nc.scalar.mul(Sn1n[:], cs["SnatF"][:], -1.0)
            nc.scalar.mul(Cn2[:], cs["CnatF"][:], 1.0 / 65536.0)
            nc.scalar.mul(Sn2[:], cs["SnatF"][:], 1.0 / 65536.0)

            for i in range(n_planes):
                # ---- loads ----
                x_sb = small.tile([128, 128], F32R, tag="x_sb")
                nc.sync.dma_start(x_sb[:], xs_t[i])
                kt_sb = small.tile([KS, KS], F32R, tag="kt_sb")
                nc.sync.dma_start(kt_sb[:], kt_t[i])

                # xcd[k, 2j+b] = x[k, j]  (column-doubled x for the xu matmul)
                xcd = small.tile([128, 256], F32R, tag="xcd")
                nc.scalar.copy(
                    xcd[:].rearrange("p (f b) -> p f b", b=2),
                    x_sb[:].unsqueeze(2).broadcast_to([128, 128, 2]),
                )

                # ---- fft128: Z then G=[gr|gs] ----
                pz = psS.tile([128, 256], F32, tag="pz")
                nc.tensor.matmul(pz[:], x_sb[:], cs["F1cs"][:], start=True, stop=True)
                z_sb = small.tile([128, 256], F32R, tag="z_sb")
                nc.scalar.copy(z_sb[:], pz[:])

                pg = psS.tile([128, 256], F32, tag="pz")
                nc.tensor.matmul(pg[:], z_sb[:, 0:128], cs["F1cs"][:], start=True, stop=False)
                nc.tensor.matmul(pg[:], z_sb[:, 128:256], cs["F1b"][:], start=False, stop=True)
                g_sb = small.tile([128, 256], BF16, tag="g_sb")
                nc.scalar.copy(g_sb[:], pg[:])

                # ---- A = k [GcT|GsT]; A2 = k [GDa|GDb] ----
                pa = psA.tile([KS, 512], F32, tag="pa")
                nc.tensor.matmul(pa[:], kt_sb[:], cs["GG"][:], start=True, stop=True)
                a_sb = small.tile([KS, 512], F32R, tag="a_sb")
                nc.scalar.copy(a_sb[:], pa[:])

                pa2 = psA.tile([KS, 512], F32, tag="pa")
                nc.tensor.matmul(pa2[:, 0:256], kt_sb[:], cs["GDa"][:], start=True, stop=True)
                nc.tensor.matmul(pa2[:, 256:512], kt_sb[:], cs["GDb"][:], start=True, stop=True)
                a2_sb = small.tile([KS, 512], F32R, tag="a2_sb")
                nc.vector.tensor_copy(a2_sb[:], pa2[:])

                # ---- Y = Gd k Gd^T = [Yr|Ys] ----
                py = psS.tile([128, 256], F32, tag="pz")
                nc.tensor.matmul(py[:], cs["GDa"][:, 0:128], a2_sb[:, 0:256], start=True, stop=False)
                nc.tensor.matmul(py[:], cs["GDa"][:, 128:256], a2_sb[:, 256:512], start=False, stop=True)
                ysb = small.tile([128, 256], F32, tag="ysb")
                nc.scalar.copy(ysb[:], py[:])

                # ---- FB halves -> fb_sb [128,1024] bf16: flat = c*512 + hb*256 + f ----
                fb_sb = big.tile([128, 1024], BF16, tag="fb_sb")
                fbv = fb_sb[:].rearrange("p (c hb f) -> p hb c f", c=2, hb=2)
                for hb in range(2):
                    hsl = slice(hb * 128, (hb + 1) * 128)
                    pfb = psB.tile([128, 512], F32, tag="pfb")
                    nc.tensor.matmul(pfb[:, 0:256], cs["GG"][:, hsl], a_sb[:, 0:256], start=True, stop=False)
                    nc.tensor.matmul(pfb[:, 0:256], cs["GsTn"][:, hsl], a_sb[:, 256:512], start=False, stop=True)
                    nc.tensor.matmul(pfb[:, 256:512], cs["GG"][:, hsl], a_sb[:, 256:512], start=True, stop=False)
                    nc.tensor.matmul(pfb[:, 256:512], cs["GG"][:, 256 + hb * 128:256 + (hb + 1) * 128], a_sb[:, 0:256], start=False, stop=True)
                    nc.scalar.copy(fbv[:, hb], pfb[:].rearrange("p (c f) -> p c f", c=2))

                # ---- W = blocksum |FB|^2 ----
                sq_sb = big.tile([128, 1024], BF16, tag="sq_sb")
                nc.vector.tensor_tensor(sq_sb[:], fb_sb[:], fb_sb[:], MULT)
                W_sb = small.tile([128, 128], F32, tag="W_sb")
                nc.vector.tensor_reduce(
                    W_sb[:], sq_sb[:].rearrange("p (g f) -> p f g", g=8),
                    mybir.AxisListType.X, ADD,
                )

                # ---- wt = (4-Y)/(W+4be); gw = g*wt  (Pool + DVE recip) ----
                den = small.tile([128, 128], F32, tag="den")
                nc.gpsimd.tensor_tensor(den[:], W_sb[:], be4sb[:, i:i + 1].broadcast_to([128, 128]), ADD)
                dinv = small.tile([128, 128], F32, tag="dinv")
                nc.vector.reciprocal_approx_fast(dinv[:], den[:])
                wt4 = small.tile([128, 128], F32, tag="wt4")
                nc.vector.tensor_scalar(wt4[:], ysb[:, 0:128], -1.0, 4.0, MULT, ADD)
                wtr = small.tile([128, 128], BF16, tag="wtr")
                nc.gpsimd.tensor_tensor(wtr[:], wt4[:], dinv[:], MULT)
                # wtsn = Ys*dinv = -wts (sign folded into the gw combines below)
                wtsn = small.tile([128, 128], BF16, tag="wtsn")
                nc.gpsimd.tensor_tensor(wtsn[:], ysb[:, 128:256], dinv[:], MULT)

                gq1 = small.tile([128, 128], BF16, tag="gq1")
                gq2 = small.tile([128, 128], BF16, tag="gq2")
                gq3 = small.tile([128, 128], BF16, tag="gq3")
                gq4 = small.tile([128, 128], BF16, tag="gq4")
                gw = small.tile([128, 256], BF16, tag="gw")
                nc.gpsimd.tensor_tensor(gq1[:], g_sb[:, 0:128], wtr[:], MULT)
                nc.gpsimd.tensor_tensor(gq2[:], g_sb[:, 128:256], wtsn[:], MULT)
                nc.gpsimd.tensor_tensor(gw[:, 0:128], gq1[:], gq2[:], ADD)
                nc.gpsimd.tensor_tensor(gq3[:], g_sb[:, 0:128], wtsn[:], MULT)
                nc.gpsimd.tensor_tensor(gq4[:], g_sb[:, 128:256], wtr[:], MULT)
                nc.gpsimd.tensor_tensor(gw[:, 128:256], gq4[:], gq3[:], SUB)

                # ---- FX = conj(FB)*tile(gw): Qa = fb*rep8(gwr), Qb = fb*rep8(gws) ----
                Qa = big.tile([128, 1024], BF16, tag="Qa")
                Qb = big.tile([128, 1024], BF16, tag="Qb")
                fb8 = fb_sb[:].rearrange("p (g f) -> p g f", g=8)
                nc.vector.tensor_tensor(Qa[:].rearrange("p (g f) -> p g f", g=8), fb8,
                                        gw[:, 0:128].unsqueeze(1).broadcast_to([128, 8, 128]), MULT)
                nc.vector.tensor_tensor(Qb[:].rearrange("p (g f) -> p g f", g=8), fb8,
                                        gw[:, 128:256].unsqueeze(1).broadcast_to([128, 8, 128]), MULT)
                fxr = big.tile([128, 512], BF16, tag="fxr")
                fxs = big.tile([128, 512], BF16, tag="fxs")
                nc.vector.tensor_tensor(fxr[:], Qa[:, 0:512], Qb[:, 512:1024], ADD)
                nc.vector.tensor_tensor(fxs[:], Qb[:, 0:512], Qa[:, 512:1024], SUB)

                # ---- ifft stage 1 (bf16): vt_sb [128,1024]: flat = c*512 + fb*256 + y ----
                vt_sb = big.tile([128, 1024], F32R, tag="vt_sb")
                vtv = vt_sb[:].rearrange("p (c fb f) -> p fb c f", c=2, fb=2)
                for fbi in range(2):
                    pvt = psB.tile([128, 512], F32, tag="pvt")
                    # one accumulation group at a time per psum bank
                    for kc in range(2):
                        lsl = slice(kc * 256 + fbi * 128, kc * 256 + (fbi + 1) * 128)
                        csl = slice(kc * 256, (kc + 1) * 256)
                        nc.tensor.matmul(pvt[:, 0:256], fxr[:, lsl], Cn1[:, csl], start=(kc == 0), stop=False)
                        nc.tensor.matmul(pvt[:, 0:256], fxs[:, lsl], Sn1[:, csl], start=False, stop=(kc == 1))
                    for kc in range(2):
                        lsl = slice(kc * 256 + fbi * 128, kc * 256 + (fbi + 1) * 128)
                        csl = slice(kc * 256, (kc + 1) * 256)
                        nc.tensor.matmul(pvt[:, 256:512], fxs[:, lsl], Cn1[:, csl], start=(kc == 0), stop=False)
                        nc.tensor.matmul(pvt[:, 256:512], fxr[:, lsl], Sn1n[:, csl], start=False, stop=(kc == 1))
                    nc.scalar.copy(vtv[:, fbi], pvt[:].rearrange("p (c f) -> p c f", c=2))

                # ---- ifft stage 2 (fp32r, scaled) + xu matmul ----
                po = psO.tile([128, 512], F32, tag="po")
                for mb in range(2):
                    osl = slice(mb * 256, (mb + 1) * 256)
                    for fbi in range(2):
                        vr = slice(fbi * 256 + mb * 128, fbi * 256 + (mb + 1) * 128)
                        vs = slice(512 + fbi * 256 + mb * 128, 512 + fbi * 256 + (mb + 1) * 128)
                        csl = slice(fbi * 256, (fbi + 1) * 256)
                        nc.tensor.matmul(po[:, osl], vt_sb[:, vr], Cn2[:, csl], start=(fbi == 0), stop=False)
                        nc.tensor.matmul(po[:, osl], vt_sb[:, vs], Sn2[:, csl], start=False, stop=False)
                    nc.tensor.matmul(po[:, osl], cs["Prow0"][:] if mb == 0 else cs["Prow1"][:], xcd[:], start=False, stop=True)

                out_sb = big.tile([128, 512], F32, tag="out_sb")
                nc.scalar.copy(out_sb[:], po[:])
                nc.sync.dma_start(
                    out_t[i].rearrange("(hb p) f -> p hb f", p=128),
                    out_sb[:].rearrange("p (hb f) -> p hb f", hb=2),
                )

    nc.compile()
    return nc


@functools.lru_cache(maxsize=2)
def _built(n_planes=NPL):
    return build_nc(n_planes)


def make_in_maps(x, k, alpha, n_planes=NPL, n_cores=N_CORES):
    consts = _host_consts()
    alpha_c = alpha.reshape(-1).astype(np.float64)  # [64]
    be = (1.0 / (1.0 + np.exp(-(alpha_c - 9.0))) + 1e-3).astype(np.float32)
    cpc = n_planes // 4  # channels per core
    in_maps = []
    for ci in range(n_cores):
        chs = slice(ci * cpc, (ci + 1) * cpc)
        xs = np.ascontiguousarray(x[:, chs].transpose(1, 0, 2, 3).reshape(n_planes, 128, 128))
        kt = np.ascontiguousarray(k[:, chs].transpose(1, 0, 3, 2).reshape(n_planes, KS, KS))
        be_pl = np.repeat(be[chs], 4)  # plane order: (c_loc, b)
        be4 = np.broadcast_to(4.0 * be_pl, (128, n_planes)).astype(np.float32).copy()
        m = {"xs": xs, "kt": kt, "be4": be4}
        m.update(consts)
        in_maps.append(m)
    return in_maps


def kernel(x, k, alpha, sf=2, **_ignored):
    x = np.asarray(x, dtype=np.float32)
    k = np.asarray(k, dtype=np.float32)
    alpha = np.asarray(alpha, dtype=np.float32)
    assert int(sf) == 2 and x.shape == (4, 64, 128, 128) and k.shape == (4, 64, KS, KS)

    nc = _built(NPL)
    in_maps = make_in_maps(x, k, alpha)
    res = run_bass_kernel_spmd(nc, in_maps, core_ids=list(range(N_CORES)))
    out = np.empty((4, 64, 256, 256), np.float32)
    cpc = NPL // 4
    for ci in range(N_CORES):
        o = res.results[ci]["out"].reshape(cpc, 4, 256, 256).transpose(1, 0, 2, 3)
        out[:, ci * cpc:(ci + 1) * cpc] = o
    return out


if __name__ == "__main__":
    rng = np.random.default_rng(0)
    x = rng.standard_normal((4, 64, 128, 128), dtype=np.float32)
    k = rng.random((4, 64, KS, KS), dtype=np.float32)
    alpha = np.zeros((1, 64, 1, 1), np.float32)
    out = kernel(x, k, alpha, 2)
    print("out", out.shape, out.dtype, float(np.abs(out).max()))


# revision 10
# speedup vs baseline: 2.3869x; 1.0815x over previous
"""Trainium2 Bass kernel for ConvReverseDataNet (USRNet-style FFT data step), v2.

Math per (b,c) plane (sf=2), storage convention X = Xr - i*Xs:
  g   = fft2_128(x)
  FB  = G k G^T, G = F256[:, roll_idx]            (256x256, as [128,1024] r|s)
  W   = blocksum|FB|^2 (128x128); Y0 = Gd k Gd^T  (= blocksum(FB*DD))
  wt  = (4 - Y0) / (W + 4*be)
  gw  = g * wt
  FX  = conj(FB) * tile(gw)
  out = real(ifft2_256(FX)) + nn_upsample(x)      (upsample via matmul w/ Prow)
where Gd[u0,:] = sum_a d_a[u0] * G[a*128+u0,:], d = 1+exp(-2pi i t/256) halves.

Engine split: PE fp32r matmuls (ifft stage1 in bf16), DVE bf16 elementwise
(2x_1p), Act PSUM->SBUF copies, GpSimd(Pool) wt/gw small ops (SBUF only).
Sharding: 256 (b,c) planes over 8 cores; core ci gets channels ci*8..ci*8+7.
"""

import functools
import sys

import numpy as np

if "/opt/trn_rl_repo" not in sys.path:
    sys.path.insert(0, "/opt/trn_rl_repo")

from concourse import bacc, bass, mybir, tile  # noqa: E402
from concourse.bass_utils import run_bass_kernel_spmd  # noqa: E402

F32 = mybir.dt.float32
F32R = mybir.dt.float32r
BF16 = mybir.dt.bfloat16
MULT = mybir.AluOpType.mult
ADD = mybir.AluOpType.add
SUB = mybir.AluOpType.subtract

N_CORES = 8
NPL = 32  # planes per core
KS = 25


def _host_consts():
    t1 = np.arange(128)
    th1 = 2 * np.pi * np.outer(t1, t1) / 128
    C1 = np.cos(th1)
    S1 = np.sin(th1)
    t2 = np.arange(256)
    th2 = 2 * np.pi * np.outer(t2, t2) / 256
    C2 = np.cos(th2)
    S2 = np.sin(th2)
    idx = (np.arange(KS) - (KS // 2)) % 256
    GcT = C2[idx, :]  # [25,256]
    GsT = S2[idx, :]
    th = 2 * np.pi * t1 / 256
    dr0 = 1 + np.cos(th)
    ds0 = np.sin(th)
    dr1 = 1 - np.cos(th)
    ds1 = -np.sin(th)
    Gc0, Gc1 = GcT[:, :128].T, GcT[:, 128:].T
    Gs0, Gs1 = GsT[:, :128].T, GsT[:, 128:].T
    Gdr = dr0[:, None] * Gc0 - ds0[:, None] * Gs0 + dr1[:, None] * Gc1 - ds1[:, None] * Gs1
    Gds = dr0[:, None] * Gs0 + ds0[:, None] * Gc0 + dr1[:, None] * Gs1 + ds1[:, None] * Gc1
    Cnat = C2.reshape(2, 128, 256).transpose(1, 0, 2).reshape(128, 512)
    Snat = S2.reshape(2, 128, 256).transpose(1, 0, 2).reshape(128, 512)
    Prow = np.zeros((2, 128, 128))
    for mb in range(2):
        for m in range(128):
            Prow[mb, mb * 64 + m // 2, m] = 1.0
    f32 = lambda a: np.ascontiguousarray(a, dtype=np.float32)
    return {
        "F1cs": f32(np.concatenate([C1, S1], 1)),       # [128,256]
        "F1b": f32(np.concatenate([-S1, C1], 1)),       # [128,256]
        "GG": f32(np.concatenate([GcT, GsT], 1)),       # [25,512]
        "GsTn": f32(-GsT),                              # [25,256]
        "GDa": f32(np.concatenate([Gdr.T, Gds.T], 1)),  # [25,256]
        "GDb": f32(np.concatenate([-Gds.T, Gdr.T], 1)),  # [25,256]
        "CnatF": f32(Cnat),                             # [128,512]
        "SnatF": f32(Snat),                             # [128,512]
        "Prow0": f32(Prow[0]),                          # [128,128]
        "Prow1": f32(Prow[1]),                          # [128,128]
    }


CONST_SHAPES = {
    "F1cs": [128, 256], "F1b": [128, 256],
    "GG": [25, 512], "GsTn": [25, 256],
    "GDa": [25, 256], "GDb": [25, 256],
    "CnatF": [128, 512], "SnatF": [128, 512],
    "Prow0": [128, 128], "Prow1": [128, 128],
}


def build_nc(n_planes=NPL):
    nc = bacc.Bacc("TRN2", target_bir_lowering=False, debug=False, num_devices=N_CORES)

    xs_t = nc.dram_tensor("xs", [n_planes, 128, 128], F32R, kind="ExternalInput")
    kt_t = nc.dram_tensor("kt", [n_planes, KS, KS], F32R, kind="ExternalInput")
    be4_t = nc.dram_tensor("be4", [128, n_planes], F32, kind="ExternalInput")
    const_t = {n: nc.dram_tensor(n, s, (F32 if n in ("CnatF", "SnatF") else F32R), kind="ExternalInput") for n, s in CONST_SHAPES.items()}
    out_t = nc.dram_tensor("out", [n_planes, 256, 256], F32, kind="ExternalOutput")

    with tile.TileContext(nc) as tc:
        with (
            tc.tile_pool(name="cpool", bufs=1) as cpool,
            tc.tile_pool(name="small", bufs=3) as small,
            tc.tile_pool(name="big", bufs=2) as big,
            tc.tile_pool(name="psS", bufs=2, space="PSUM") as psS,   # [128,256] Z/G/Y
            tc.tile_pool(name="psA", bufs=1, space="PSUM") as psA,   # [25,512] A/A2
            tc.tile_pool(name="psB", bufs=2, space="PSUM") as psB,   # [128,512] FB/VT
            tc.tile_pool(name="psO", bufs=1, space="PSUM") as psO,   # [128,512] po
        ):
            cs = {}
            for n, s in CONST_SHAPES.items():
                cs[n] = cpool.tile(s, (F32 if n in ("CnatF", "SnatF") else F32R), tag=n, name=f"c_{n}")
                nc.sync.dma_start(cs[n][:], const_t[n][:])
            be4sb = cpool.tile([128, n_planes], F32, tag="be4sb")
            nc.sync.dma_start(be4sb[:], be4_t[:])

            # preamble: derived consts (bf16 stage-1 DFT; scaled f32 stage-2 DFT)
            Cn1 = cpool.tile([128, 512], BF16, tag="Cn1")
            Sn1 = cpool.tile([128, 512], BF16, tag="Sn1")
            Sn1n = cpool.tile([128, 512], BF16, tag="Sn1n")
            Cn2 = cpool.tile([128, 512], F32R, tag="Cn2")
            Sn2 = cpool.tile([128, 512], F32R, tag="Sn2")
            nc.scalar.copy(Cn1[:], cs["CnatF"][:])
            nc.scalar.copy(Sn1[:], cs["SnatF"][:])
            nc.scalar.mul(Sn1n[:], cs["SnatF"][:], -1.0)
            nc.scalar.mul(Cn2[:], cs["CnatF"][:], 1.0 / 65536.0)
            nc.scalar.mul(Sn2[:], cs["SnatF"][:], 1.0 / 65536.0)
            Cn2n = cpool.tile([128, 512], F32R, tag="Cn2n")
            nc.scalar.mul(Cn2n[:], cs["CnatF"][:], -1.0 / 65536.0)

            qa_t, qb_t, xcd_t = [None, None], [None, None], [None, None]
            for i in range(n_planes):
                # ---- loads ----
                x_sb = small.tile([128, 128], F32R, tag="x_sb")
                nc.sync.dma_start(x_sb[:], xs_t[i])
                kt_sb = small.tile([KS, KS], F32R, tag="kt_sb")
                nc.sync.dma_start(kt_sb[:], kt_t[i])

                # xcd[k, 2j+b] = x[k, j]  (column-doubled x for the xu matmul)
                xcd = small.tile([128, 256], F32R, tag="xcd")
                nc.scalar.copy(
                    xcd[:].rearrange("p (f b) -> p f b", b=2),
                    x_sb[:].unsqueeze(2).broadcast_to([128, 128, 2]),
                )

                # ---- fft128: Z then G=[gr|gs] ----
                pz = psS.tile([128, 256], F32, tag="pz")
                nc.tensor.matmul(pz[:], x_sb[:], cs["F1cs"][:], start=True, stop=True)
                z_sb = small.tile([128, 256], F32R, tag="z_sb")
                nc.scalar.copy(z_sb[:], pz[:])

                pg = psS.tile([128, 256], F32, tag="pz")
                nc.tensor.matmul(pg[:], z_sb[:, 0:128], cs["F1cs"][:], start=True, stop=False)
                nc.tensor.matmul(pg[:], z_sb[:, 128:256], cs["F1b"][:], start=False, stop=True)
                g_sb = small.tile([128, 256], BF16, tag="g_sb")
                nc.scalar.copy(g_sb[:], pg[:])

                # ---- A = k [GcT|GsT]; A2 = k [GDa|GDb] ----
                pa = psA.tile([KS, 512], F32, tag="pa")
                nc.tensor.matmul(pa[:], kt_sb[:], cs["GG"][:], start=True, stop=True)
                a_sb = small.tile([KS, 512], F32R, tag="a_sb")
                nc.scalar.copy(a_sb[:], pa[:])

                pa2 = psA.tile([KS, 512], F32, tag="pa")
                nc.tensor.matmul(pa2[:, 0:256], kt_sb[:], cs["GDa"][:], start=True, stop=True)
                nc.tensor.matmul(pa2[:, 256:512], kt_sb[:], cs["GDb"][:], start=True, stop=True)
                a2_sb = small.tile([KS, 512], F32R, tag="a2_sb")
                nc.vector.tensor_copy(a2_sb[:], pa2[:])

                # ---- Y = Gd k Gd^T = [Yr|Ys] ----
                py = psS.tile([128, 256], F32, tag="pz")
                nc.tensor.matmul(py[:], cs["GDa"][:, 0:128], a2_sb[:, 0:256], start=True, stop=False)
                nc.tensor.matmul(py[:], cs["GDa"][:, 128:256], a2_sb[:, 256:512], start=False, stop=True)
                ysb = small.tile([128, 256], F32, tag="ysb")
                nc.scalar.copy(ysb[:], py[:])

                # ---- FB halves -> fb_sb [128,1024] bf16: flat = c*512 + hb*256 + f ----
                fb_sb = big.tile([128, 1024], BF16, tag="fb_sb")
                fbv = fb_sb[:].rearrange("p (c hb f) -> p hb c f", c=2, hb=2)
                for hb in range(2):
                    hsl = slice(hb * 128, (hb + 1) * 128)
                    pfb = psB.tile([128, 512], F32, tag="pfb")
                    nc.tensor.matmul(pfb[:, 0:256], cs["GG"][:, hsl], a_sb[:, 0:256], start=True, stop=False)
                    nc.tensor.matmul(pfb[:, 0:256], cs["GsTn"][:, hsl], a_sb[:, 256:512], start=False, stop=True)
                    nc.tensor.matmul(pfb[:, 256:512], cs["GG"][:, hsl], a_sb[:, 256:512], start=True, stop=False)
                    nc.tensor.matmul(pfb[:, 256:512], cs["GG"][:, 256 + hb * 128:256 + (hb + 1) * 128], a_sb[:, 0:256], start=False, stop=True)
                    nc.scalar.copy(fbv[:, hb], pfb[:].rearrange("p (c f) -> p c f", c=2))

                # ---- W = blocksum |FB|^2 ----
                sq_sb = big.tile([128, 1024], BF16, tag="sq_sb")
                nc.vector.tensor_tensor(sq_sb[:], fb_sb[:], fb_sb[:], MULT)
                W_sb = small.tile([128, 128], F32, tag="W_sb")
                nc.vector.tensor_reduce(
                    W_sb[:], sq_sb[:].rearrange("p (g f) -> p f g", g=8),
                    mybir.AxisListType.X, ADD,
                )

                # ---- wt = (4-Y)/(W+4be); gw = g*wt  (Pool + DVE recip) ----
                den = small.tile([128, 128], F32, tag="den")
                nc.gpsimd.tensor_tensor(den[:], W_sb[:], be4sb[:, i:i + 1].broadcast_to([128, 128]), ADD)
                dinv = small.tile([128, 128], F32, tag="dinv")
                nc.vector.reciprocal_approx_fast(dinv[:], den[:])
                wt4 = small.tile([128, 128], F32, tag="wt4")
                nc.vector.tensor_scalar(wt4[:], ysb[:, 0:128], -1.0, 4.0, MULT, ADD)
                wtr = small.tile([128, 128], BF16, tag="wtr")
                nc.gpsimd.tensor_tensor(wtr[:], wt4[:], dinv[:], MULT)
                # wtsn = Ys*dinv = -wts (sign folded into the gw combines below)
                wtsn = small.tile([128, 128], BF16, tag="wtsn")
                nc.gpsimd.tensor_tensor(wtsn[:], ysb[:, 128:256], dinv[:], MULT)

                gq1 = small.tile([128, 128], BF16, tag="gq1")
                gq2 = small.tile([128, 128], BF16, tag="gq2")
                gq3 = small.tile([128, 128], BF16, tag="gq3")
                gq4 = small.tile([128, 128], BF16, tag="gq4")
                gw = small.tile([128, 256], BF16, tag="gw")
                nc.gpsimd.tensor_tensor(gq1[:], g_sb[:, 0:128], wtr[:], MULT)
                nc.gpsimd.tensor_tensor(gq2[:], g_sb[:, 128:256], wtsn[:], MULT)
                nc.gpsimd.tensor_tensor(gw[:, 0:128], gq1[:], gq2[:], ADD)
                nc.gpsimd.tensor_tensor(gq3[:], g_sb[:, 0:128], wtsn[:], MULT)
                nc.gpsimd.tensor_tensor(gq4[:], g_sb[:, 128:256], wtr[:], MULT)
                nc.gpsimd.tensor_tensor(gw[:, 128:256], gq4[:], gq3[:], SUB)

                # ---- FX = conj(FB)*tile(gw): Qa = fb*rep8(gwr), Qb = fb*rep8(gws) ----
                Qa = big.tile([128, 1024], BF16, tag="Qa")
                Qb = big.tile([128, 1024], BF16, tag="Qb")
                fb8 = fb_sb[:].rearrange("p (g f) -> p g f", g=8)
                nc.vector.tensor_tensor(Qa[:].rearrange("p (g f) -> p g f", g=8), fb8,
                                        gw[:, 0:128].unsqueeze(1).broadcast_to([128, 8, 128]), MULT)
                nc.vector.tensor_tensor(Qb[:].rearrange("p (g f) -> p g f", g=8), fb8,
                                        gw[:, 128:256].unsqueeze(1).broadcast_to([128, 8, 128]), MULT)
                j = i % 2
                qa_t[j], qb_t[j], xcd_t[j] = Qa, Qb, xcd
                if j == 0:
                    continue

                # ---- pair-packed complex ifft: FXc = FX0 + i*FX1 ----
                # FX0r = Qa0[:, :512]+Qb0[:, 512:]; FX0s = Qb0[:, :512]-Qa0[:, 512:]
                # FX1 likewise; FXCr = FX0r+FX1s, FXCs = FX0s-FX1r
                t0 = big.tile([128, 512], BF16, tag="t0")
                t1 = big.tile([128, 512], BF16, tag="t1")
                t2 = big.tile([128, 512], BF16, tag="t2")
                t3 = big.tile([128, 512], BF16, tag="t3")
                fxr = big.tile([128, 512], BF16, tag="fxr")
                fxs = big.tile([128, 512], BF16, tag="fxs")
                nc.vector.tensor_tensor(t0[:], qa_t[0][:, 0:512], qb_t[0][:, 512:1024], ADD)
                nc.vector.tensor_tensor(t1[:], qb_t[1][:, 0:512], qa_t[1][:, 512:1024], SUB)
                nc.vector.tensor_tensor(fxr[:], t0[:], t1[:], ADD)
                nc.vector.tensor_tensor(t2[:], qb_t[0][:, 0:512], qa_t[0][:, 512:1024], SUB)
                nc.vector.tensor_tensor(t3[:], qa_t[1][:, 0:512], qb_t[1][:, 512:1024], ADD)
                nc.vector.tensor_tensor(fxs[:], t2[:], t3[:], SUB)

                # ---- ifft stage 1 (bf16): vt_sb [128,1024]: flat = c*512 + fb*256 + y ----
                vt_sb = big.tile([128, 1024], F32R, tag="vt_sb")
                vtv = vt_sb[:].rearrange("p (c fb f) -> p fb c f", c=2, fb=2)
                for fbi in range(2):
                    pvt = psB.tile([128, 512], F32, tag="pvt")
                    # one accumulation group at a time per psum bank
                    for kc in range(2):
                        lsl = slice(kc * 256 + fbi * 128, kc * 256 + (fbi + 1) * 128)
                        csl = slice(kc * 256, (kc + 1) * 256)
                        nc.tensor.matmul(pvt[:, 0:256], fxr[:, lsl], Cn1[:, csl], start=(kc == 0), stop=False)
                        nc.tensor.matmul(pvt[:, 0:256], fxs[:, lsl], Sn1[:, csl], start=False, stop=(kc == 1))
                    for kc in range(2):
                        lsl = slice(kc * 256 + fbi * 128, kc * 256 + (fbi + 1) * 128)
                        csl = slice(kc * 256, (kc + 1) * 256)
                        nc.tensor.matmul(pvt[:, 256:512], fxs[:, lsl], Cn1[:, csl], start=(kc == 0), stop=False)
                        nc.tensor.matmul(pvt[:, 256:512], fxr[:, lsl], Sn1n[:, csl], start=False, stop=(kc == 1))
                    nc.scalar.copy(vtv[:, fbi], pvt[:].rearrange("p (c f) -> p c f", c=2))

                # ---- ifft stage 2 (fp32r, scaled): real -> plane 2pr, imag -> 2pr+1 ----
                for j2 in range(2):
                    po = psO.tile([128, 512], F32, tag="po")
                    ca, cb = (Cn2, Sn2) if j2 == 0 else (Sn2, Cn2n)
                    for mb in range(2):
                        osl = slice(mb * 256, (mb + 1) * 256)
                        for fbi in range(2):
                            vr = slice(fbi * 256 + mb * 128, fbi * 256 + (mb + 1) * 128)
                            vs = slice(512 + fbi * 256 + mb * 128, 512 + fbi * 256 + (mb + 1) * 128)
                            csl = slice(fbi * 256, (fbi + 1) * 256)
                            nc.tensor.matmul(po[:, osl], vt_sb[:, vr], ca[:, csl], start=(fbi == 0), stop=False)
                            nc.tensor.matmul(po[:, osl], vt_sb[:, vs], cb[:, csl], start=False, stop=False)
                        nc.tensor.matmul(po[:, osl], cs["Prow0"][:] if mb == 0 else cs["Prow1"][:], xcd_t[j2][:], start=False, stop=True)

                    out_sb = big.tile([128, 512], F32, tag="out_sb")
                    nc.scalar.copy(out_sb[:], po[:])
                    nc.sync.dma_start(
                        out_t[i - 1 + j2].rearrange("(hb p) f -> p hb f", p=128),
                        out_sb[:].rearrange("p (hb f) -> p hb f", hb=2),
                    )

    nc.compile()
    return nc


@functools.lru_cache(maxsize=2)
def _built(n_planes=NPL):
    return build_nc(n_planes)


def make_in_maps(x, k, alpha, n_planes=NPL, n_cores=N_CORES):
    consts = _host_consts()
    alpha_c = alpha.reshape(-1).astype(np.float64)  # [64]
    be = (1.0 / (1.0 + np.exp(-(alpha_c - 9.0))) + 1e-3).astype(np.float32)
    cpc = n_planes // 4  # channels per core
    in_maps = []
    for ci in range(n_cores):
        chs = slice(ci * cpc, (ci + 1) * cpc)
        xs = np.ascontiguousarray(x[:, chs].transpose(1, 0, 2, 3).reshape(n_planes, 128, 128))
        kt = np.ascontiguousarray(k[:, chs].transpose(1, 0, 3, 2).reshape(n_planes, KS, KS))
        be_pl = np.repeat(be[chs], 4)  # plane order: (c_loc, b)
        be4 = np.broadcast_to(4.0 * be_pl, (128, n_planes)).astype(np.float32).copy()
        m = {"xs": xs, "kt": kt, "be4": be4}
        m.update(consts)
        in_maps.append(m)
    return in_maps


def kernel(x, k, alpha, sf=2, **_ignored):
    x = np.asarray(x, dtype=np.float32)
    k = np.asarray(k, dtype=np.float32)
    alpha = np.asarray(alpha, dtype=np.float32)
    assert int(sf) == 2 and x.shape == (4, 64, 128, 128) and k.shape == (4, 64, KS, KS)

    nc = _built(NPL)
    in_maps = make_in_maps(x, k, alpha)
    res = run_bass_kernel_spmd(nc, in_maps, core_ids=list(range(N_CORES)))
    out = np.empty((4, 64, 256, 256), np.float32)
    cpc = NPL // 4
    for ci in range(N_CORES):
        o = res.results[ci]["out"].reshape(cpc, 4, 256, 256).transpose(1, 0, 2, 3)
        out[:, ci * cpc:(ci + 1) * cpc] = o
    return out


if __name__ == "__main__":
    rng = np.random.default_rng(0)
    x = rng.standard_normal((4, 64, 128, 128), dtype=np.float32)
    k = rng.random((4, 64, KS, KS), dtype=np.float32)
    alpha = np.zeros((1, 64, 1, 1), np.float32)
    out = kernel(x, k, alpha, 2)
    print("out", out.shape, out.dtype, float(np.abs(out).max()))


# revision 12
# speedup vs baseline: 2.4034x; 1.0069x over previous
"""Trainium2 Bass kernel for ConvReverseDataNet (USRNet-style FFT data step), v2.

Math per (b,c) plane (sf=2), storage convention X = Xr - i*Xs:
  g   = fft2_128(x)
  FB  = G k G^T, G = F256[:, roll_idx]            (256x256, as [128,1024] r|s)
  W   = blocksum|FB|^2 (128x128); Y0 = Gd k Gd^T  (= blocksum(FB*DD))
  wt  = (4 - Y0) / (W + 4*be)
  gw  = g * wt
  FX  = conj(FB) * tile(gw)
  out = real(ifft2_256(FX)) + nn_upsample(x)      (upsample via matmul w/ Prow)
where Gd[u0,:] = sum_a d_a[u0] * G[a*128+u0,:], d = 1+exp(-2pi i t/256) halves.

Engine split: PE fp32r matmuls (ifft stage1 in bf16), DVE bf16 elementwise
(2x_1p), Act PSUM->SBUF copies, GpSimd(Pool) wt/gw small ops (SBUF only).
Sharding: 256 (b,c) planes over 8 cores; core ci gets channels ci*8..ci*8+7.
"""

import functools
import sys

import numpy as np

if "/opt/trn_rl_repo" not in sys.path:
    sys.path.insert(0, "/opt/trn_rl_repo")

from concourse import bacc, bass, mybir, tile  # noqa: E402
from concourse.bass_utils import run_bass_kernel_spmd  # noqa: E402

F32 = mybir.dt.float32
F32R = mybir.dt.float32r
BF16 = mybir.dt.bfloat16
MULT = mybir.AluOpType.mult
ADD = mybir.AluOpType.add
SUB = mybir.AluOpType.subtract

N_CORES = 8
NPL = 32  # planes per core
KS = 25


def _host_consts():
    t1 = np.arange(128)
    th1 = 2 * np.pi * np.outer(t1, t1) / 128
    C1 = np.cos(th1)
    S1 = np.sin(th1)
    t2 = np.arange(256)
    th2 = 2 * np.pi * np.outer(t2, t2) / 256
    C2 = np.cos(th2)
    S2 = np.sin(th2)
    idx = (np.arange(KS) - (KS // 2)) % 256
    GcT = C2[idx, :]  # [25,256]
    GsT = S2[idx, :]
    th = 2 * np.pi * t1 / 256
    dr0 = 1 + np.cos(th)
    ds0 = np.sin(th)
    dr1 = 1 - np.cos(th)
    ds1 = -np.sin(th)
    Gc0, Gc1 = GcT[:, :128].T, GcT[:, 128:].T
    Gs0, Gs1 = GsT[:, :128].T, GsT[:, 128:].T
    Gdr = dr0[:, None] * Gc0 - ds0[:, None] * Gs0 + dr1[:, None] * Gc1 - ds1[:, None] * Gs1
    Gds = dr0[:, None] * Gs0 + ds0[:, None] * Gc0 + dr1[:, None] * Gs1 + ds1[:, None] * Gc1
    Cnat = C2.reshape(2, 128, 256).transpose(1, 0, 2).reshape(128, 512)
    Snat = S2.reshape(2, 128, 256).transpose(1, 0, 2).reshape(128, 512)
    Prow = np.zeros((2, 128, 128))
    for mb in range(2):
        for m in range(128):
            Prow[mb, mb * 64 + m // 2, m] = 1.0
    f32 = lambda a: np.ascontiguousarray(a, dtype=np.float32)
    return {
        "F1cs": f32(np.concatenate([C1, S1], 1)),       # [128,256]
        "F1b": f32(np.concatenate([-S1, C1], 1)),       # [128,256]
        "GG": f32(np.concatenate([GcT, GsT], 1)),       # [25,512]
        "GsTn": f32(-GsT),                              # [25,256]
        "GDa": f32(np.concatenate([Gdr.T, Gds.T], 1)),  # [25,256]
        "GDb": f32(np.concatenate([-Gds.T, Gdr.T], 1)),  # [25,256]
        "GDab": f32(np.concatenate([Gdr.T, Gds.T, -Gds.T, Gdr.T], 1)),  # [25,512]
        "CnatF": f32(Cnat),                             # [128,512]
        "SnatF": f32(Snat),                             # [128,512]
        "Prow0": f32(Prow[0]),                          # [128,128]
        "Prow1": f32(Prow[1]),                          # [128,128]
    }


CONST_SHAPES = {
    "F1cs": [128, 256], "F1b": [128, 256],
    "GG": [25, 512], "GsTn": [25, 256],
    "GDa": [25, 256], "GDb": [25, 256], "GDab": [25, 512],
    "CnatF": [128, 512], "SnatF": [128, 512],
    "Prow0": [128, 128], "Prow1": [128, 128],
}


def build_nc(n_planes=NPL):
    nc = bacc.Bacc("TRN2", target_bir_lowering=False, debug=False, num_devices=N_CORES)

    xs_t = nc.dram_tensor("xs", [n_planes, 128, 128], F32R, kind="ExternalInput")
    kt_t = nc.dram_tensor("kt", [n_planes, KS, KS], F32R, kind="ExternalInput")
    be4_t = nc.dram_tensor("be4", [128, n_planes], F32, kind="ExternalInput")
    const_t = {n: nc.dram_tensor(n, s, (F32 if n in ("CnatF", "SnatF") else F32R), kind="ExternalInput") for n, s in CONST_SHAPES.items()}
    out_t = nc.dram_tensor("out", [n_planes, 256, 256], F32, kind="ExternalOutput")

    with tile.TileContext(nc) as tc:
        with (
            tc.tile_pool(name="cpool", bufs=1) as cpool,
            tc.tile_pool(name="small", bufs=3) as small,
            tc.tile_pool(name="big", bufs=2) as big,
            tc.tile_pool(name="psS", bufs=2, space="PSUM") as psS,   # [128,256] Z/G/Y
            tc.tile_pool(name="psA", bufs=1, space="PSUM") as psA,   # [25,512] A/A2
            tc.tile_pool(name="psB", bufs=2, space="PSUM") as psB,   # [128,512] FB/VT
            tc.tile_pool(name="psO", bufs=1, space="PSUM") as psO,   # [128,512] po
        ):
            cs = {}
            for n, s in CONST_SHAPES.items():
                cs[n] = cpool.tile(s, (F32 if n in ("CnatF", "SnatF") else F32R), tag=n, name=f"c_{n}")
                nc.sync.dma_start(cs[n][:], const_t[n][:])
            be4sb = cpool.tile([128, n_planes], F32, tag="be4sb")
            nc.sync.dma_start(be4sb[:], be4_t[:])

            # preamble: derived consts (bf16 stage-1 DFT; scaled f32 stage-2 DFT)
            Cn1 = cpool.tile([128, 512], BF16, tag="Cn1")
            Sn1 = cpool.tile([128, 512], BF16, tag="Sn1")
            Sn1n = cpool.tile([128, 512], BF16, tag="Sn1n")
            Cn2 = cpool.tile([128, 512], F32R, tag="Cn2")
            Sn2 = cpool.tile([128, 512], F32R, tag="Sn2")
            nc.scalar.copy(Cn1[:], cs["CnatF"][:])
            nc.scalar.copy(Sn1[:], cs["SnatF"][:])
            nc.scalar.mul(Sn1n[:], cs["SnatF"][:], -1.0)
            nc.scalar.mul(Cn2[:], cs["CnatF"][:], 1.0 / 65536.0)
            nc.scalar.mul(Sn2[:], cs["SnatF"][:], 1.0 / 65536.0)
            Cn2n = cpool.tile([128, 512], F32R, tag="Cn2n")
            nc.scalar.mul(Cn2n[:], cs["CnatF"][:], -1.0 / 65536.0)

            qa_t, qb_t, xcd_t = [None, None], [None, None], [None, None]
            pending_stage2 = None
            for i in range(n_planes):
                # ---- loads ----
                x_sb = small.tile([128, 128], F32R, tag="x_sb")
                nc.sync.dma_start(x_sb[:], xs_t[i])
                kt_sb = small.tile([KS, KS], F32R, tag="kt_sb")
                nc.sync.dma_start(kt_sb[:], kt_t[i])

                # xcd[k, 2j+b] = x[k, j]  (column-doubled x for the xu matmul)
                xcd = small.tile([128, 256], F32R, tag="xcd", bufs=5)
                nc.scalar.copy(
                    xcd[:].rearrange("p (f b) -> p f b", b=2),
                    x_sb[:].unsqueeze(2).broadcast_to([128, 128, 2]),
                )

                # ---- fft128: Z then G=[gr|gs] ----
                pz = psS.tile([128, 256], F32, tag="pz")
                nc.tensor.matmul(pz[:], x_sb[:], cs["F1cs"][:], start=True, stop=True)
                z_sb = small.tile([128, 256], F32R, tag="z_sb")
                nc.scalar.copy(z_sb[:], pz[:])

                pg = psS.tile([128, 256], F32, tag="pz")
                nc.tensor.matmul(pg[:], z_sb[:, 0:128], cs["F1cs"][:], start=True, stop=False)
                nc.tensor.matmul(pg[:], z_sb[:, 128:256], cs["F1b"][:], start=False, stop=True)
                g_sb = small.tile([128, 256], BF16, tag="g_sb")
                nc.scalar.copy(g_sb[:], pg[:])

                # ---- A = k [GcT|GsT]; A2 = k [GDa|GDb] ----
                pa = psA.tile([KS, 512], F32, tag="pa")
                nc.tensor.matmul(pa[:], kt_sb[:], cs["GG"][:], start=True, stop=True)
                a_sb = small.tile([KS, 512], F32R, tag="a_sb")
                nc.scalar.copy(a_sb[:], pa[:])

                pa2 = psA.tile([KS, 512], F32, tag="pa")
                nc.tensor.matmul(pa2[:], kt_sb[:], cs["GDab"][:], start=True, stop=True)
                a2_sb = small.tile([KS, 512], F32R, tag="a2_sb")
                nc.vector.tensor_copy(a2_sb[:], pa2[:])

                # ---- Y = Gd k Gd^T = [Yr|Ys] ----
                py = psS.tile([128, 256], F32, tag="pz")
                nc.tensor.matmul(py[:], cs["GDa"][:, 0:128], a2_sb[:, 0:256], start=True, stop=False)
                nc.tensor.matmul(py[:], cs["GDa"][:, 128:256], a2_sb[:, 256:512], start=False, stop=True)
                ysb = small.tile([128, 256], F32, tag="ysb")
                nc.scalar.copy(ysb[:], py[:])

                # ---- FB halves -> fb_sb [128,1024] bf16: flat = c*512 + hb*256 + f ----
                fb_sb = big.tile([128, 1024], BF16, tag="fb_sb")
                fbv = fb_sb[:].rearrange("p (c hb f) -> p hb c f", c=2, hb=2)
                for hb in range(2):
                    hsl = slice(hb * 128, (hb + 1) * 128)
                    pfb = psB.tile([128, 512], F32, tag="pfb")
                    nc.tensor.matmul(pfb[:, 0:256], cs["GG"][:, hsl], a_sb[:, 0:256], start=True, stop=False)
                    nc.tensor.matmul(pfb[:, 0:256], cs["GsTn"][:, hsl], a_sb[:, 256:512], start=False, stop=True)
                    nc.tensor.matmul(pfb[:, 256:512], cs["GG"][:, hsl], a_sb[:, 256:512], start=True, stop=False)
                    nc.tensor.matmul(pfb[:, 256:512], cs["GG"][:, 256 + hb * 128:256 + (hb + 1) * 128], a_sb[:, 0:256], start=False, stop=True)
                    nc.scalar.copy(fbv[:, hb], pfb[:].rearrange("p (c f) -> p c f", c=2))

                # ---- W = blocksum |FB|^2 ----
                sq_sb = big.tile([128, 1024], BF16, tag="sq_sb")
                nc.vector.tensor_tensor(sq_sb[:], fb_sb[:], fb_sb[:], MULT)
                W_sb = small.tile([128, 128], F32, tag="W_sb")
                nc.vector.tensor_reduce(
                    W_sb[:], sq_sb[:].rearrange("p (g f) -> p f g", g=8),
                    mybir.AxisListType.X, ADD,
                )

                # ---- wt = (4-Y)/(W+4be); gw = g*wt  (Pool + DVE recip) ----
                den = small.tile([128, 128], F32, tag="den")
                nc.gpsimd.tensor_tensor(den[:], W_sb[:], be4sb[:, i:i + 1].broadcast_to([128, 128]), ADD)
                dinv = small.tile([128, 128], F32, tag="dinv")
                nc.vector.reciprocal_approx_fast(dinv[:], den[:])
                wt4 = small.tile([128, 128], F32, tag="wt4")
                nc.vector.tensor_scalar(wt4[:], ysb[:, 0:128], -1.0, 4.0, MULT, ADD)
                wtr = small.tile([128, 128], BF16, tag="wtr")
                nc.gpsimd.tensor_tensor(wtr[:], wt4[:], dinv[:], MULT)
                # wtsn = Ys*dinv = -wts (sign folded into the gw combines below)
                wtsn = small.tile([128, 128], BF16, tag="wtsn")
                nc.gpsimd.tensor_tensor(wtsn[:], ysb[:, 128:256], dinv[:], MULT)

                gq1 = small.tile([128, 128], BF16, tag="gq1")
                gq2 = small.tile([128, 128], BF16, tag="gq2")
                gq3 = small.tile([128, 128], BF16, tag="gq3")
                gq4 = small.tile([128, 128], BF16, tag="gq4")
                gw = small.tile([128, 256], BF16, tag="gw")
                nc.gpsimd.tensor_tensor(gq1[:], g_sb[:, 0:128], wtr[:], MULT)
                nc.gpsimd.tensor_tensor(gq2[:], g_sb[:, 128:256], wtsn[:], MULT)
                nc.gpsimd.tensor_tensor(gw[:, 0:128], gq1[:], gq2[:], ADD)
                nc.gpsimd.tensor_tensor(gq3[:], g_sb[:, 0:128], wtsn[:], MULT)
                nc.gpsimd.tensor_tensor(gq4[:], g_sb[:, 128:256], wtr[:], MULT)
                nc.gpsimd.tensor_tensor(gw[:, 128:256], gq4[:], gq3[:], SUB)

                # ---- FX = conj(FB)*tile(gw): Qa = fb*rep8(gwr), Qb = fb*rep8(gws) ----
                Qa = big.tile([128, 1024], BF16, tag="Qa")
                Qb = big.tile([128, 1024], BF16, tag="Qb")
                fb8 = fb_sb[:].rearrange("p (g f) -> p g f", g=8)
                nc.vector.tensor_tensor(Qa[:].rearrange("p (g f) -> p g f", g=8), fb8,
                                        gw[:, 0:128].unsqueeze(1).broadcast_to([128, 8, 128]), MULT)
                nc.vector.tensor_tensor(Qb[:].rearrange("p (g f) -> p g f", g=8), fb8,
                                        gw[:, 128:256].unsqueeze(1).broadcast_to([128, 8, 128]), MULT)
                j = i % 2
                qa_t[j], qb_t[j], xcd_t[j] = Qa, Qb, xcd
                if j == 0:
                    continue

                # ---- pair-packed complex ifft: FXc = FX0 + i*FX1 ----
                # FX0r = Qa0[:, :512]+Qb0[:, 512:]; FX0s = Qb0[:, :512]-Qa0[:, 512:]
                # FX1 likewise; FXCr = FX0r+FX1s, FXCs = FX0s-FX1r
                t0 = big.tile([128, 512], BF16, tag="t0")
                t1 = big.tile([128, 512], BF16, tag="t1")
                t2 = big.tile([128, 512], BF16, tag="t2")
                t3 = big.tile([128, 512], BF16, tag="t3")
                fxr = big.tile([128, 512], BF16, tag="fxr")
                fxs = big.tile([128, 512], BF16, tag="fxs")
                nc.vector.tensor_tensor(t0[:], qa_t[0][:, 0:512], qb_t[0][:, 512:1024], ADD)
                nc.vector.tensor_tensor(t1[:], qb_t[1][:, 0:512], qa_t[1][:, 512:1024], SUB)
                nc.vector.tensor_tensor(fxr[:], t0[:], t1[:], ADD)
                nc.vector.tensor_tensor(t2[:], qb_t[0][:, 0:512], qa_t[0][:, 512:1024], SUB)
                nc.vector.tensor_tensor(t3[:], qa_t[1][:, 0:512], qb_t[1][:, 512:1024], ADD)
                nc.vector.tensor_tensor(fxs[:], t2[:], t3[:], SUB)

                # ---- ifft stage 1 (bf16): vt_sb [128,1024]: flat = c*512 + fb*256 + y ----
                vt_sb = big.tile([128, 1024], F32R, tag="vt_sb")
                vtv = vt_sb[:].rearrange("p (c fb f) -> p fb c f", c=2, fb=2)
                for fbi in range(2):
                    pvt = psB.tile([128, 512], F32, tag="pvt")
                    # one accumulation group at a time per psum bank
                    for kc in range(2):
                        lsl = slice(kc * 256 + fbi * 128, kc * 256 + (fbi + 1) * 128)
                        csl = slice(kc * 256, (kc + 1) * 256)
                        nc.tensor.matmul(pvt[:, 0:256], fxr[:, lsl], Cn1[:, csl], start=(kc == 0), stop=False)
                        nc.tensor.matmul(pvt[:, 0:256], fxs[:, lsl], Sn1[:, csl], start=False, stop=(kc == 1))
                    for kc in range(2):
                        lsl = slice(kc * 256 + fbi * 128, kc * 256 + (fbi + 1) * 128)
                        csl = slice(kc * 256, (kc + 1) * 256)
                        nc.tensor.matmul(pvt[:, 256:512], fxs[:, lsl], Cn1[:, csl], start=(kc == 0), stop=False)
                        nc.tensor.matmul(pvt[:, 256:512], fxr[:, lsl], Sn1n[:, csl], start=False, stop=(kc == 1))
                    nc.scalar.copy(vtv[:, fbi], pvt[:].rearrange("p (c f) -> p c f", c=2))

                # ---- ifft stage 2 (fp32r, scaled): deferred one pair for overlap ----
                def make_stage2(vt_sb=vt_sb, xcds=tuple(xcd_t), base=i - 1):
                    def emit():
                        for j2 in range(2):
                            po = psO.tile([128, 512], F32, tag="po")
                            ca, cb = (Cn2, Sn2) if j2 == 0 else (Sn2, Cn2n)
                            for mb in range(2):
                                osl = slice(mb * 256, (mb + 1) * 256)
                                for fbi in range(2):
                                    vr = slice(fbi * 256 + mb * 128, fbi * 256 + (mb + 1) * 128)
                                    vs = slice(512 + fbi * 256 + mb * 128, 512 + fbi * 256 + (mb + 1) * 128)
                                    csl = slice(fbi * 256, (fbi + 1) * 256)
                                    nc.tensor.matmul(po[:, osl], vt_sb[:, vr], ca[:, csl], start=(fbi == 0), stop=False)
                                    nc.tensor.matmul(po[:, osl], vt_sb[:, vs], cb[:, csl], start=False, stop=False)
                                nc.tensor.matmul(po[:, osl], cs["Prow0"][:] if mb == 0 else cs["Prow1"][:], xcds[j2][:], start=False, stop=True)
                            out_sb = big.tile([128, 512], F32, tag="out_sb")
                            nc.scalar.copy(out_sb[:], po[:])
                            nc.sync.dma_start(
                                out_t[base + j2].rearrange("(hb p) f -> p hb f", p=128),
                                out_sb[:].rearrange("p (hb f) -> p hb f", hb=2),
                            )
                    return emit
                if pending_stage2 is not None:
                    pending_stage2_new = make_stage2()
                    pending_stage2()
                    pending_stage2 = pending_stage2_new
                else:
                    pending_stage2 = make_stage2()
            pending_stage2()

    nc.compile()
    return nc


@functools.lru_cache(maxsize=2)
def _built(n_planes=NPL):
    return build_nc(n_planes)


def make_in_maps(x, k, alpha, n_planes=NPL, n_cores=N_CORES):
    consts = _host_consts()
    alpha_c = alpha.reshape(-1).astype(np.float64)  # [64]
    be = (1.0 / (1.0 + np.exp(-(alpha_c - 9.0))) + 1e-3).astype(np.float32)
    cpc = n_planes // 4  # channels per core
    in_maps = []
    for ci in range(n_cores):
        chs = slice(ci * cpc, (ci + 1) * cpc)
        xs = np.ascontiguousarray(x[:, chs].transpose(1, 0, 2, 3).reshape(n_planes, 128, 128))
        kt = np.ascontiguousarray(k[:, chs].transpose(1, 0, 3, 2).reshape(n_planes, KS, KS))
        be_pl = np.repeat(be[chs], 4)  # plane order: (c_loc, b)
        be4 = np.broadcast_to(4.0 * be_pl, (128, n_planes)).astype(np.float32).copy()
        m = {"xs": xs, "kt": kt, "be4": be4}
        m.update(consts)
        in_maps.append(m)
    return in_maps


def kernel(x, k, alpha, sf=2, **_ignored):
    x = np.asarray(x, dtype=np.float32)
    k = np.asarray(k, dtype=np.float32)
    alpha = np.asarray(alpha, dtype=np.float32)
    assert int(sf) == 2 and x.shape == (4, 64, 128, 128) and k.shape == (4, 64, KS, KS)

    nc = _built(NPL)
    in_maps = make_in_maps(x, k, alpha)
    res = run_bass_kernel_spmd(nc, in_maps, core_ids=list(range(N_CORES)))
    out = np.empty((4, 64, 256, 256), np.float32)
    cpc = NPL // 4
    for ci in range(N_CORES):
        o = res.results[ci]["out"].reshape(cpc, 4, 256, 256).transpose(1, 0, 2, 3)
        out[:, ci * cpc:(ci + 1) * cpc] = o
    return out


if __name__ == "__main__":
    rng = np.random.default_rng(0)
    x = rng.standard_normal((4, 64, 128, 128), dtype=np.float32)
    k = rng.random((4, 64, KS, KS), dtype=np.float32)
    alpha = np.zeros((1, 64, 1, 1), np.float32)
    out = kernel(x, k, alpha, 2)
    print("out", out.shape, out.dtype, float(np.abs(out).max()))


# revision 13
# speedup vs baseline: 2.4645x; 1.0254x over previous
"""Trainium2 Bass kernel for ConvReverseDataNet (USRNet-style FFT data step), v2.

Math per (b,c) plane (sf=2), storage convention X = Xr - i*Xs:
  g   = fft2_128(x)
  FB  = G k G^T, G = F256[:, roll_idx]            (256x256, as [128,1024] r|s)
  W   = blocksum|FB|^2 (128x128); Y0 = Gd k Gd^T  (= blocksum(FB*DD))
  wt  = (4 - Y0) / (W + 4*be)
  gw  = g * wt
  FX  = conj(FB) * tile(gw)
  out = real(ifft2_256(FX)) + nn_upsample(x)      (upsample via matmul w/ Prow)
where Gd[u0,:] = sum_a d_a[u0] * G[a*128+u0,:], d = 1+exp(-2pi i t/256) halves.

Engine split: PE fp32r matmuls (ifft stage1 in bf16), DVE bf16 elementwise
(2x_1p), Act PSUM->SBUF copies, GpSimd(Pool) wt/gw small ops (SBUF only).
Sharding: 256 (b,c) planes over 8 cores; core ci gets channels ci*8..ci*8+7.
"""

import functools
import sys

import numpy as np

if "/opt/trn_rl_repo" not in sys.path:
    sys.path.insert(0, "/opt/trn_rl_repo")

from concourse import bacc, bass, mybir, tile  # noqa: E402
from concourse.bass_utils import run_bass_kernel_spmd  # noqa: E402

F32 = mybir.dt.float32
F32R = mybir.dt.float32r
BF16 = mybir.dt.bfloat16
MULT = mybir.AluOpType.mult
ADD = mybir.AluOpType.add
SUB = mybir.AluOpType.subtract

N_CORES = 8
NPL = 32  # planes per core
KS = 25


def _host_consts():
    t1 = np.arange(128)
    th1 = 2 * np.pi * np.outer(t1, t1) / 128
    C1 = np.cos(th1)
    S1 = np.sin(th1)
    t2 = np.arange(256)
    th2 = 2 * np.pi * np.outer(t2, t2) / 256
    C2 = np.cos(th2)
    S2 = np.sin(th2)
    idx = (np.arange(KS) - (KS // 2)) % 256
    GcT = C2[idx, :]  # [25,256]
    GsT = S2[idx, :]
    th = 2 * np.pi * t1 / 256
    dr0 = 1 + np.cos(th)
    ds0 = np.sin(th)
    dr1 = 1 - np.cos(th)
    ds1 = -np.sin(th)
    Gc0, Gc1 = GcT[:, :128].T, GcT[:, 128:].T
    Gs0, Gs1 = GsT[:, :128].T, GsT[:, 128:].T
    Gdr = dr0[:, None] * Gc0 - ds0[:, None] * Gs0 + dr1[:, None] * Gc1 - ds1[:, None] * Gs1
    Gds = dr0[:, None] * Gs0 + ds0[:, None] * Gc0 + dr1[:, None] * Gs1 + ds1[:, None] * Gc1
    Cnat = C2.reshape(2, 128, 256).transpose(1, 0, 2).reshape(128, 512)
    Snat = S2.reshape(2, 128, 256).transpose(1, 0, 2).reshape(128, 512)
    Prow = np.zeros((2, 128, 128))
    for mb in range(2):
        for m in range(128):
            Prow[mb, mb * 64 + m // 2, m] = 1.0
    f32 = lambda a: np.ascontiguousarray(a, dtype=np.float32)
    return {
        "F1cs": f32(np.concatenate([C1, S1], 1)),       # [128,256]
        "F1b": f32(np.concatenate([-S1, C1], 1)),       # [128,256]
        "GG": f32(np.concatenate([GcT, GsT], 1)),       # [25,512]
        "GsTn": f32(-GsT),                              # [25,256]
        "GDa": f32(np.concatenate([Gdr.T, Gds.T], 1)),  # [25,256]
        "GDb": f32(np.concatenate([-Gds.T, Gdr.T], 1)),  # [25,256]
        "GDab": f32(np.concatenate([Gdr.T, Gds.T, -Gds.T, Gdr.T], 1)),  # [25,512]
        "CnatF": f32(Cnat),                             # [128,512]
        "SnatF": f32(Snat),                             # [128,512]
        "Prow0": f32(Prow[0]),                          # [128,128]
        "Prow1": f32(Prow[1]),                          # [128,128]
    }


CONST_SHAPES = {
    "F1cs": [128, 256], "F1b": [128, 256],
    "GG": [25, 512], "GsTn": [25, 256],
    "GDa": [25, 256], "GDb": [25, 256], "GDab": [25, 512],
    "CnatF": [128, 512], "SnatF": [128, 512],
    "Prow0": [128, 128], "Prow1": [128, 128],
}


def build_nc(n_planes=NPL):
    nc = bacc.Bacc("TRN2", target_bir_lowering=False, debug=False, num_devices=N_CORES)

    xs_t = nc.dram_tensor("xs", [n_planes, 128, 128], F32R, kind="ExternalInput")
    kt_t = nc.dram_tensor("kt", [n_planes, KS, KS], F32R, kind="ExternalInput")
    be4_t = nc.dram_tensor("be4", [128, n_planes], F32, kind="ExternalInput")
    const_t = {n: nc.dram_tensor(n, s, (F32 if n in ("CnatF", "SnatF") else F32R), kind="ExternalInput") for n, s in CONST_SHAPES.items()}
    out_t = nc.dram_tensor("out", [n_planes, 256, 256], F32, kind="ExternalOutput")

    with tile.TileContext(nc) as tc:
        with (
            tc.tile_pool(name="cpool", bufs=1) as cpool,
            tc.tile_pool(name="small", bufs=4) as small,
            tc.tile_pool(name="big", bufs=3) as big,
            tc.tile_pool(name="psS", bufs=2, space="PSUM") as psS,   # [128,256] Z/G/Y
            tc.tile_pool(name="psA", bufs=1, space="PSUM") as psA,   # [25,512] A/A2
            tc.tile_pool(name="psB", bufs=2, space="PSUM") as psB,   # [128,512] FB/VT
            tc.tile_pool(name="psO", bufs=1, space="PSUM") as psO,   # [128,512] po
        ):
            cs = {}
            for n, s in CONST_SHAPES.items():
                cs[n] = cpool.tile(s, (F32 if n in ("CnatF", "SnatF") else F32R), tag=n, name=f"c_{n}")
                nc.sync.dma_start(cs[n][:], const_t[n][:])
            be4sb = cpool.tile([128, n_planes], F32, tag="be4sb")
            nc.sync.dma_start(be4sb[:], be4_t[:])

            # preamble: derived consts (bf16 stage-1 DFT; scaled f32 stage-2 DFT)
            Cn1 = cpool.tile([128, 512], BF16, tag="Cn1")
            Sn1 = cpool.tile([128, 512], BF16, tag="Sn1")
            Sn1n = cpool.tile([128, 512], BF16, tag="Sn1n")
            Cn2 = cpool.tile([128, 512], F32R, tag="Cn2")
            Sn2 = cpool.tile([128, 512], F32R, tag="Sn2")
            nc.scalar.copy(Cn1[:], cs["CnatF"][:])
            nc.scalar.copy(Sn1[:], cs["SnatF"][:])
            nc.scalar.mul(Sn1n[:], cs["SnatF"][:], -1.0)
            nc.scalar.mul(Cn2[:], cs["CnatF"][:], 1.0 / 65536.0)
            nc.scalar.mul(Sn2[:], cs["SnatF"][:], 1.0 / 65536.0)
            Cn2n = cpool.tile([128, 512], F32R, tag="Cn2n")
            nc.scalar.mul(Cn2n[:], cs["CnatF"][:], -1.0 / 65536.0)

            qa_t, qb_t, xcd_t = [None, None], [None, None], [None, None]
            pending_stage2 = None
            for i in range(n_planes):
                # ---- loads ----
                x_sb = small.tile([128, 128], F32R, tag="x_sb")
                nc.sync.dma_start(x_sb[:], xs_t[i])
                kt_sb = small.tile([KS, KS], F32R, tag="kt_sb")
                nc.sync.dma_start(kt_sb[:], kt_t[i])

                # xcd[k, 2j+b] = x[k, j]  (column-doubled x for the xu matmul)
                xcd = small.tile([128, 256], F32R, tag="xcd", bufs=5)
                nc.scalar.copy(
                    xcd[:].rearrange("p (f b) -> p f b", b=2),
                    x_sb[:].unsqueeze(2).broadcast_to([128, 128, 2]),
                )

                # ---- fft128: Z then G=[gr|gs] ----
                pz = psS.tile([128, 256], F32, tag="pz")
                nc.tensor.matmul(pz[:], x_sb[:], cs["F1cs"][:], start=True, stop=True)
                z_sb = small.tile([128, 256], F32R, tag="z_sb")
                nc.scalar.copy(z_sb[:], pz[:])

                pg = psS.tile([128, 256], F32, tag="pz")
                nc.tensor.matmul(pg[:], z_sb[:, 0:128], cs["F1cs"][:], start=True, stop=False)
                nc.tensor.matmul(pg[:], z_sb[:, 128:256], cs["F1b"][:], start=False, stop=True)
                g_sb = small.tile([128, 256], BF16, tag="g_sb")
                nc.scalar.copy(g_sb[:], pg[:])

                # ---- A = k [GcT|GsT]; A2 = k [GDa|GDb] ----
                pa = psA.tile([KS, 512], F32, tag="pa")
                nc.tensor.matmul(pa[:], kt_sb[:], cs["GG"][:], start=True, stop=True)
                a_sb = small.tile([KS, 512], F32R, tag="a_sb")
                nc.scalar.copy(a_sb[:], pa[:])

                pa2 = psA.tile([KS, 512], F32, tag="pa")
                nc.tensor.matmul(pa2[:], kt_sb[:], cs["GDab"][:], start=True, stop=True)
                a2_sb = small.tile([KS, 512], F32R, tag="a2_sb")
                nc.vector.tensor_copy(a2_sb[:], pa2[:])

                # ---- Y = Gd k Gd^T = [Yr|Ys] ----
                py = psS.tile([128, 256], F32, tag="pz")
                nc.tensor.matmul(py[:], cs["GDa"][:, 0:128], a2_sb[:, 0:256], start=True, stop=False)
                nc.tensor.matmul(py[:], cs["GDa"][:, 128:256], a2_sb[:, 256:512], start=False, stop=True)
                ysb = small.tile([128, 256], F32, tag="ysb")
                nc.scalar.copy(ysb[:], py[:])

                # ---- FB halves -> fb_sb [128,1024] bf16: flat = c*512 + hb*256 + f ----
                fb_sb = big.tile([128, 1024], BF16, tag="fb_sb")
                fbv = fb_sb[:].rearrange("p (c hb f) -> p hb c f", c=2, hb=2)
                for hb in range(2):
                    hsl = slice(hb * 128, (hb + 1) * 128)
                    pfb = psB.tile([128, 512], F32, tag="pfb")
                    nc.tensor.matmul(pfb[:, 0:256], cs["GG"][:, hsl], a_sb[:, 0:256], start=True, stop=False)
                    nc.tensor.matmul(pfb[:, 0:256], cs["GsTn"][:, hsl], a_sb[:, 256:512], start=False, stop=True)
                    nc.tensor.matmul(pfb[:, 256:512], cs["GG"][:, hsl], a_sb[:, 256:512], start=True, stop=False)
                    nc.tensor.matmul(pfb[:, 256:512], cs["GG"][:, 256 + hb * 128:256 + (hb + 1) * 128], a_sb[:, 0:256], start=False, stop=True)
                    nc.scalar.copy(fbv[:, hb], pfb[:].rearrange("p (c f) -> p c f", c=2))

                # ---- W = blocksum |FB|^2 ----
                sq_sb = big.tile([128, 1024], BF16, tag="sq_sb")
                nc.vector.tensor_tensor(sq_sb[:], fb_sb[:], fb_sb[:], MULT)
                W_sb = small.tile([128, 128], F32, tag="W_sb")
                nc.vector.tensor_reduce(
                    W_sb[:], sq_sb[:].rearrange("p (g f) -> p f g", g=8),
                    mybir.AxisListType.X, ADD,
                )

                # ---- wt = (4-Y)/(W+4be); gw = g*wt  (Pool + DVE recip) ----
                den = small.tile([128, 128], F32, tag="den")
                nc.gpsimd.tensor_tensor(den[:], W_sb[:], be4sb[:, i:i + 1].broadcast_to([128, 128]), ADD)
                dinv = small.tile([128, 128], F32, tag="dinv")
                nc.vector.reciprocal_approx_fast(dinv[:], den[:])
                wt4 = small.tile([128, 128], F32, tag="wt4")
                nc.vector.tensor_scalar(wt4[:], ysb[:, 0:128], -1.0, 4.0, MULT, ADD)
                wtr = small.tile([128, 128], BF16, tag="wtr")
                nc.gpsimd.tensor_tensor(wtr[:], wt4[:], dinv[:], MULT)
                # wtsn = Ys*dinv = -wts (sign folded into the gw combines below)
                wtsn = small.tile([128, 128], BF16, tag="wtsn")
                nc.gpsimd.tensor_tensor(wtsn[:], ysb[:, 128:256], dinv[:], MULT)

                gq1 = small.tile([128, 128], BF16, tag="gq1")
                gq2 = small.tile([128, 128], BF16, tag="gq2")
                gq3 = small.tile([128, 128], BF16, tag="gq3")
                gq4 = small.tile([128, 128], BF16, tag="gq4")
                gw = small.tile([128, 256], BF16, tag="gw")
                nc.gpsimd.tensor_tensor(gq1[:], g_sb[:, 0:128], wtr[:], MULT)
                nc.gpsimd.tensor_tensor(gq2[:], g_sb[:, 128:256], wtsn[:], MULT)
                nc.gpsimd.tensor_tensor(gw[:, 0:128], gq1[:], gq2[:], ADD)
                nc.gpsimd.tensor_tensor(gq3[:], g_sb[:, 0:128], wtsn[:], MULT)
                nc.gpsimd.tensor_tensor(gq4[:], g_sb[:, 128:256], wtr[:], MULT)
                nc.gpsimd.tensor_tensor(gw[:, 128:256], gq4[:], gq3[:], SUB)

                # ---- FX = conj(FB)*tile(gw): Qa = fb*rep8(gwr), Qb = fb*rep8(gws) ----
                Qa = big.tile([128, 1024], BF16, tag="Qa")
                Qb = big.tile([128, 1024], BF16, tag="Qb")
                fb8 = fb_sb[:].rearrange("p (g f) -> p g f", g=8)
                nc.vector.tensor_tensor(Qa[:].rearrange("p (g f) -> p g f", g=8), fb8,
                                        gw[:, 0:128].unsqueeze(1).broadcast_to([128, 8, 128]), MULT)
                nc.vector.tensor_tensor(Qb[:].rearrange("p (g f) -> p g f", g=8), fb8,
                                        gw[:, 128:256].unsqueeze(1).broadcast_to([128, 8, 128]), MULT)
                j = i % 2
                qa_t[j], qb_t[j], xcd_t[j] = Qa, Qb, xcd
                if j == 0:
                    continue

                # ---- pair-packed complex ifft: FXc = FX0 + i*FX1 ----
                # FX0r = Qa0[:, :512]+Qb0[:, 512:]; FX0s = Qb0[:, :512]-Qa0[:, 512:]
                # FX1 likewise; FXCr = FX0r+FX1s, FXCs = FX0s-FX1r
                t0 = big.tile([128, 512], BF16, tag="t0")
                t1 = big.tile([128, 512], BF16, tag="t1")
                t2 = big.tile([128, 512], BF16, tag="t2")
                t3 = big.tile([128, 512], BF16, tag="t3")
                fxr = big.tile([128, 512], BF16, tag="fxr")
                fxs = big.tile([128, 512], BF16, tag="fxs")
                nc.vector.tensor_tensor(t0[:], qa_t[0][:, 0:512], qb_t[0][:, 512:1024], ADD)
                nc.vector.tensor_tensor(t1[:], qb_t[1][:, 0:512], qa_t[1][:, 512:1024], SUB)
                nc.vector.tensor_tensor(fxr[:], t0[:], t1[:], ADD)
                nc.vector.tensor_tensor(t2[:], qb_t[0][:, 0:512], qa_t[0][:, 512:1024], SUB)
                nc.vector.tensor_tensor(t3[:], qa_t[1][:, 0:512], qb_t[1][:, 512:1024], ADD)
                nc.vector.tensor_tensor(fxs[:], t2[:], t3[:], SUB)

                # ---- ifft stage 1 (bf16): vt_sb [128,1024]: flat = c*512 + fb*256 + y ----
                vt_sb = big.tile([128, 1024], F32R, tag="vt_sb")
                vtv = vt_sb[:].rearrange("p (c fb f) -> p fb c f", c=2, fb=2)
                for fbi in range(2):
                    pvt = psB.tile([128, 512], F32, tag="pvt")
                    # one accumulation group at a time per psum bank
                    for kc in range(2):
                        lsl = slice(kc * 256 + fbi * 128, kc * 256 + (fbi + 1) * 128)
                        csl = slice(kc * 256, (kc + 1) * 256)
                        nc.tensor.matmul(pvt[:, 0:256], fxr[:, lsl], Cn1[:, csl], start=(kc == 0), stop=False)
                        nc.tensor.matmul(pvt[:, 0:256], fxs[:, lsl], Sn1[:, csl], start=False, stop=(kc == 1))
                    for kc in range(2):
                        lsl = slice(kc * 256 + fbi * 128, kc * 256 + (fbi + 1) * 128)
                        csl = slice(kc * 256, (kc + 1) * 256)
                        nc.tensor.matmul(pvt[:, 256:512], fxs[:, lsl], Cn1[:, csl], start=(kc == 0), stop=False)
                        nc.tensor.matmul(pvt[:, 256:512], fxr[:, lsl], Sn1n[:, csl], start=False, stop=(kc == 1))
                    nc.scalar.copy(vtv[:, fbi], pvt[:].rearrange("p (c f) -> p c f", c=2))

                # ---- ifft stage 2 (fp32r, scaled): deferred one pair for overlap ----
                def make_stage2(vt_sb=vt_sb, xcds=tuple(xcd_t), base=i - 1):
                    def emit():
                        for j2 in range(2):
                            po = psO.tile([128, 512], F32, tag="po")
                            ca, cb = (Cn2, Sn2) if j2 == 0 else (Sn2, Cn2n)
                            for mb in range(2):
                                osl = slice(mb * 256, (mb + 1) * 256)
                                for fbi in range(2):
                                    vr = slice(fbi * 256 + mb * 128, fbi * 256 + (mb + 1) * 128)
                                    vs = slice(512 + fbi * 256 + mb * 128, 512 + fbi * 256 + (mb + 1) * 128)
                                    csl = slice(fbi * 256, (fbi + 1) * 256)
                                    nc.tensor.matmul(po[:, osl], vt_sb[:, vr], ca[:, csl], start=(fbi == 0), stop=False)
                                    nc.tensor.matmul(po[:, osl], vt_sb[:, vs], cb[:, csl], start=False, stop=False)
                                nc.tensor.matmul(po[:, osl], cs["Prow0"][:] if mb == 0 else cs["Prow1"][:], xcds[j2][:], start=False, stop=True)
                            out_sb = big.tile([128, 512], F32, tag="out_sb")
                            nc.scalar.copy(out_sb[:], po[:])
                            nc.sync.dma_start(
                                out_t[base + j2].rearrange("(hb p) f -> p hb f", p=128),
                                out_sb[:].rearrange("p (hb f) -> p hb f", hb=2),
                            )
                    return emit
                if pending_stage2 is not None:
                    pending_stage2_new = make_stage2()
                    pending_stage2()
                    pending_stage2 = pending_stage2_new
                else:
                    pending_stage2 = make_stage2()
            pending_stage2()

    nc.compile()
    return nc


@functools.lru_cache(maxsize=2)
def _built(n_planes=NPL):
    return build_nc(n_planes)


def make_in_maps(x, k, alpha, n_planes=NPL, n_cores=N_CORES):
    consts = _host_consts()
    alpha_c = alpha.reshape(-1).astype(np.float64)  # [64]
    be = (1.0 / (1.0 + np.exp(-(alpha_c - 9.0))) + 1e-3).astype(np.float32)
    cpc = n_planes // 4  # channels per core
    in_maps = []
    for ci in range(n_cores):
        chs = slice(ci * cpc, (ci + 1) * cpc)
        xs = np.ascontiguousarray(x[:, chs].transpose(1, 0, 2, 3).reshape(n_planes, 128, 128))
        kt = np.ascontiguousarray(k[:, chs].transpose(1, 0, 3, 2).reshape(n_planes, KS, KS))
        be_pl = np.repeat(be[chs], 4)  # plane order: (c_loc, b)
        be4 = np.broadcast_to(4.0 * be_pl, (128, n_planes)).astype(np.float32).copy()
        m = {"xs": xs, "kt": kt, "be4": be4}
        m.update(consts)
        in_maps.append(m)
    return in_maps


def kernel(x, k, alpha, sf=2, **_ignored):
    x = np.asarray(x, dtype=np.float32)
    k = np.asarray(k, dtype=np.float32)
    alpha = np.asarray(alpha, dtype=np.float32)
    assert int(sf) == 2 and x.shape == (4, 64, 128, 128) and k.shape == (4, 64, KS, KS)

    nc = _built(NPL)
    in_maps = make_in_maps(x, k, alpha)
    res = run_bass_kernel_spmd(nc, in_maps, core_ids=list(range(N_CORES)))
    out = np.empty((4, 64, 256, 256), np.float32)
    cpc = NPL // 4
    for ci in range(N_CORES):
        o = res.results[ci]["out"].reshape(cpc, 4, 256, 256).transpose(1, 0, 2, 3)
        out[:, ci * cpc:(ci + 1) * cpc] = o
    return out


if __name__ == "__main__":
    rng = np.random.default_rng(0)
    x = rng.standard_normal((4, 64, 128, 128), dtype=np.float32)
    k = rng.random((4, 64, KS, KS), dtype=np.float32)
    alpha = np.zeros((1, 64, 1, 1), np.float32)
    out = kernel(x, k, alpha, 2)
    print("out", out.shape, out.dtype, float(np.abs(out).max()))
